# revision 26
# baseline (speedup 1.0000x reference)
"""Trainium2 Bass kernel for nn_BlockMoEAdapters (8 NeuronCores, SPMD).

Sharding: tokens (B*N = 4096) split contiguously across 8 cores (512 each).
Cores 0-3 hold batch 0, cores 4-7 batch 1. Attention K/V are all-gathered
(bf16, four quarter-collectives fired as soon as each producer GEMM finishes
so they hide behind the V/Q GEMMs) within each 4-core batch group; MoE
capacity ranks use a tiny 8-core all-gather of per-core expert counts.

Layout: channel-major ([channels, tokens]) on-device for all GEMMs; LayerNorm
stats via ones-matmul partition reductions (f32r for the f32 path); the LN
affine is folded into two rank-2 broadcast matmuls; softmax in [keys, tokens]
orientation with denominators accumulated via a ones-column in V, batched
into one reciprocal_approx_fast at the end of attention; weights host-retiled
into per-output-slab layouts; output shards re-transposed on host.
"""
import sys

for _p in ('/opt/trn_rl_repo',):
    if _p not in sys.path:
        sys.path.append(_p)

import ml_dtypes
import numpy as np

import concourse.bass as bass
import concourse.mybir as mybir
import concourse.tile as tile
from concourse import bacc
from concourse.bass_utils import run_bass_kernel_spmd

F32 = mybir.dt.float32
F32R = mybir.dt.float32r
F8E4 = mybir.dt.float8e4
DR = mybir.MatmulPerfMode.DoubleRow
WS = 64.0   # fp8 weight scale
BF16 = mybir.dt.bfloat16
AF = mybir.ActivationFunctionType
ALU = mybir.AluOpType

B, N, D = 2, 2048, 1024
H, HD = 16, 64
E, TOPK = 4, 2
MOEH, MLPH = 256, 4096
T = B * N
NC = 8
TL = T // NC          # 512 tokens per core
NT = TL // 128        # 4 token tiles
DT = D // 128         # 8 channel tiles
CAP = int(T * TOPK / E * 1.0)   # 2048
GRP = 4               # cores per kv-gather group
EPS = 1e-5

_cache = {}


def _mm(nc, out, lhsT, rhs, start, stop, dt=None):
    if dt is not None:
        lhsT, rhs = lhsT.bitcast(dt), rhs.bitcast(dt)
    nc.tensor.matmul(out, lhsT, rhs, start=start, stop=stop)


def _pair(ap):
    # [128, 2*X] -> [128, 2, X] for DoubleRow operands
    return ap.rearrange("p (two x) -> p two x", two=2)


def _mm8(nc, out, lhsT_pair, rhs_pair, start, stop):
    nc.tensor.matmul(out, lhsT_pair, rhs_pair, start=start, stop=stop,
                     perf_mode=DR)


def _build():
    nc = bacc.Bacc("TRN2", target_bir_lowering=False, debug=False,
                   num_devices=NC)

    def din(name, shape, dt=F32):
        return nc.dram_tensor(name, list(shape), dt, kind="ExternalInput")

    xT_d = din("xT", (D, TL), BF16)
    noiseT_d = din("noiseT", (E, TL), BF16)
    # host-retiled weight slabs (see _prep_inputs for layouts)
    wqk_d = din("wqk_l", (128, 16 * DT * 128), F8E4)
    wv_d = din("wv_l", (128, DT * 1024), BF16)
    wproj_d = din("wproj_l", (128, DT * DT * 128), BF16)
    wmlp1_d = din("wmlp1_l", (128, 32 * DT * 128), BF16)
    we1_d = din("we1_l", (128, 8 * DT * 128), F8E4)
    woutm_d = din("wout_moe", (128, DT * 8 * 128), F8E4)
    wout2_d = din("wout_mlp2", (128, DT * 32 * 128), BF16)
    wrn_d = din("wrn", (D, 2 * E), BF16)         # route cols 0:4, noise 4:8
    be2_d = din("be2", (E, D), BF16)
    lng_d = din("lng", (1, 2 * D), BF16)  # ln1_g ++ ln2_g
    lnb_d = din("lnb", (1, 2 * D), BF16)  # -(ln1_b) ++ -(ln2_b)
    bproj_d = din("bproj", (D, 1))
    brn_d = din("brn", (E, 2))      # col 0 = broute, col 1 = bnoise
    be1_d = din("be1", (E * MOEH, 1))
    bmlp1_d = din("bmlp1", (MLPH, 1))
    bmlp2_d = din("bmlp2", (D, 1))
    ones_d = din("ones128", (128, 128))
    onesb_d = din("ones128b", (128, 1), BF16)
    eye_d = din("eye128", (128, 128))
    utri_d = din("utri128", (128, 128))      # U[s,t] = 1 if s < t
    gsel_d = din("gsel", (E, E * 128), BF16)       # gsel[k, e*128+p] = (k == e)
    sel16_d = din("sel16", (2 * DT, DT * 128), BF16)  # denom selector
    wpfx_d = din("wpfx", (NC, 1))            # per-core: 1 for j < core_id

    out_d = nc.dram_tensor("out", [D, TL], F32, kind="ExternalOutput")

    rg_kv = [[0, 1, 2, 3], [4, 5, 6, 7]]
    rg_all = [list(range(NC))]

    with tile.TileContext(nc) as tc:
        with (
            tc.tile_pool(name="dram", bufs=1, space="DRAM") as dpool,
            tc.tile_pool(name="consts", bufs=1) as cpool,
            tc.tile_pool(name="persist", bufs=1) as ppool,
            tc.tile_pool(name="wslab", bufs=2) as wpool,
            tc.tile_pool(name="scratch", bufs=2) as spool,
        ):
            # ---------- collective bounce buffers (half-split kv) ----------
            k_in = [dpool.tile([D // 2, TL], F8E4, name=f"k_in{h_}")
                    for h_ in range(2)]
            v_in = [dpool.tile([128, 4 * 520], BF16, name=f"v_in{h_}")
                    for h_ in range(2)]
            k_out = [dpool.tile([GRP, D // 2, TL], F8E4, name=f"k_out{h_}")
                     for h_ in range(2)]
            v_out = [dpool.tile([GRP * 128, 4 * 520], BF16,
                                name=f"v_out{h_}") for h_ in range(2)]

            def k_in_ap(h):
                return k_in[h][:]

            def v_in_ap(h):
                return v_in[h][:]

            def k_out_ap(h, r, pq):
                return k_out[h][r, pq * 128:(pq + 1) * 128, :]

            def v_out_ap(h, r, pq):
                return v_out[h][r * 128:(r + 1) * 128,
                                pq * 520:(pq + 1) * 520]
            cnt_in = dpool.tile([1, E], F32, name="cnt_in")
            cnt_out = dpool.tile([NC, E], F32, name="cnt_out")

            # ---------- constants (gpsimd queue) ----------
            def load_const(dram, shape, dt=F32):
                t = cpool.tile(list(shape), dt, tag=dram.name, name=dram.name)
                nc.gpsimd.dma_start(t[:], dram[:])
                return t

            # urgent consts only — the rest load after the AG triggers
            onesb_sb = load_const(onesb_d, (128, 1), BF16)
            lng_sb = load_const(lng_d, (1, 2 * D), BF16)
            lnb_sb = load_const(lnb_d, (1, 2 * D), BF16)
            onesrow = cpool.tile([1, TL], BF16, tag="onesrow",
                                 name="onesrow")
            nc.vector.memset(onesrow[:], 1.0)
            epsc = cpool.tile([1, 1], F32, tag="epsc", name="epsc")
            nc.vector.memset(epsc[:], EPS)

            def load_cols(dram, n, tag):
                # [n*128, 1] dram -> sbuf [128, n] (col j = chunk j)
                t = cpool.tile([128, n], F32, tag=tag, name=tag)
                v = dram[:].rearrange("(a p) b -> a p b", p=128)
                for j in range(n):
                    nc.gpsimd.dma_start(t[:, j:j + 1], v[j])
                return t

            lateconst = {}

            def emit_late_consts():
                lateconst["ones"] = load_const(ones_d, (128, 128))
                lateconst["gsel"] = load_const(gsel_d, (E, E * 128), BF16)
                lateconst["sel16"] = load_const(sel16_d, (2 * DT, DT * 128),
                                                BF16)
                lateconst["eye"] = load_const(eye_d, (128, 128))
                lateconst["utri"] = load_const(utri_d, (128, 128))
                lateconst["wpfx"] = load_const(wpfx_d, (NC, 1))
                lateconst["brn"] = load_const(brn_d, (E, 2))
                lateconst["be2"] = load_const(be2_d, (E, D), BF16)
                wrn_sb = cpool.tile([128, DT * 2 * E], BF16, tag="wrn",
                                    name="wrn")
                for j in range(DT):
                    nc.gpsimd.dma_start(
                        wrn_sb[:, j * 2 * E:(j + 1) * 2 * E],
                        wrn_d[j * 128:(j + 1) * 128, :])
                lateconst["wrn"] = wrn_sb
                lateconst["bproj"] = load_cols(bproj_d, DT, "bproj")
                lateconst["be1"] = load_cols(be1_d, E * MOEH // 128, "be1")
                lateconst["bmlp1"] = load_cols(bmlp1_d, MLPH // 128, "bmlp1")
                lateconst["bmlp2"] = load_cols(bmlp2_d, DT, "bmlp2")

            # ---------- load x (CM, bf16 for GEMM-side, f32 kept in xres) ---
            xT_sb = []
            for j in range(DT):
                t = ppool.tile([128, TL], BF16, tag=f"xT{j}", name=f"xT{j}")
                eng = nc.sync if j % 2 == 0 else nc.gpsimd
                eng.dma_start(t[:], xT_d[j * 128:(j + 1) * 128, :])
                xT_sb.append(t)

            # ---------- LayerNorm in CM; bf16 output tiles ----------
            # out = (x * (g_r * rsig_t)) - (g_r * mu_t * rsig_t - b_r)
            # via two rank-1/2 broadcast matmuls into PSUM.
            def layernorm_cm(xtiles, lnrow, outtag, prow, pbc, opool,
                             xdt=F32, owrite=None):
                ones_col = onesb_sb[:, 0:1]
                musum = prow.tile([1, TL], F32, tag="row", name="musum")
                sqsum = prow.tile([1, TL], F32, tag="row", name="sqsum")
                for j in range(DT):
                    if xdt == BF16:
                        xb = xtiles[j]
                    else:
                        xb = spool.tile([128, TL], BF16, tag="lnxb",
                                        name="lnxb", bufs=2)
                        nc.vector.tensor_copy(xb[:], xtiles[j][:])
                    _mm(nc, musum[:], ones_col, xb[:], j == 0, j == DT - 1)
                    sq = spool.tile([128, TL], BF16, tag="lnsq", name="lnsq",
                                    bufs=2)
                    nc.vector.tensor_tensor(sq[:], xb[:], xb[:], ALU.mult)
                    _mm(nc, sqsum[:], ones_col, sq[:], j == 0, j == DT - 1)
                mu = spool.tile([1, TL], F32, tag="lnmu", name="lnmu", bufs=1)
                nc.vector.tensor_scalar_mul(mu[:], musum[:], 1.0 / D)
                msq = spool.tile([1, TL], F32, tag="lnscr", name="lnmsq",
                                 bufs=2)
                nc.vector.tensor_tensor(msq[:], mu[:], mu[:], ALU.mult)
                vare = spool.tile([1, TL], F32, tag="lnscr", name="lnvare",
                                  bufs=2)
                nc.vector.scalar_tensor_tensor(vare[:], sqsum[:], 1.0 / D,
                                               msq[:], ALU.mult, ALU.subtract)
                lnv = spool.tile([1, TL], F32, tag="lnscr", name="lnlnv",
                                 bufs=2)
                nc.scalar.activation(lnv[:], vare[:], AF.Ln,
                                     bias=epsc[0:1, 0:1])
                rsig = spool.tile([1, TL], F32, tag="lnrsig", name="lnrsig",
                                  bufs=1)
                nc.scalar.activation(rsig[:], lnv[:], AF.Exp, scale=-0.5)
                rsigb = spool.tile([1, TL], BF16, tag="lnrsigb",
                                   name="lnrsigb", bufs=1)
                nc.vector.tensor_copy(rsigb[:], rsig[:])
                murs = spool.tile([1, TL], BF16, tag="lnmurs", name="lnmurs",
                                  bufs=1)
                nc.vector.tensor_tensor(murs[:], mu[:], rsig[:], ALU.mult)
                outs = []
                for j in range(DT):
                    o0 = lnrow * D + j * 128
                    gj = lng_sb[0:1, o0:o0 + 128]
                    bj = lnb_sb[0:1, o0:o0 + 128]
                    grs_ps = pbc.tile([128, TL], F32, tag="bc", name="grs")
                    _mm(nc, grs_ps[:], gj, rsigb[:], True, True)
                    gmb_ps = pbc.tile([128, TL], F32, tag="bc", name="gmb")
                    _mm(nc, gmb_ps[:], gj, murs[:], True, False)
                    _mm(nc, gmb_ps[:], bj, onesrow[:], False, True)
                    # gmb = g*mu*rsig - b  (lnb host-negated)
                    t1 = spool.tile([128, TL], F32, tag="lnt1", name="lnt1",
                                    bufs=2)
                    nc.vector.tensor_tensor(t1[:], xtiles[j][:], grs_ps[:],
                                            ALU.mult)
                    if owrite is not None:
                        o = owrite(j)
                    else:
                        ot = opool.tile([128, TL], BF16, tag=f"{outtag}{j}",
                                        name=f"{outtag}{j}", bufs=1)
                        o = ot[:]
                        outs.append(ot)
                    nc.vector.tensor_tensor(o, t1[:], gmb_ps[:],
                                            ALU.subtract)
                return outs

            qT_sb = [ppool.tile([128, TL], F8E4, tag=f"qT{m}",
                                name=f"qT{m}") for m in range(DT)]

            with (
                tc.tile_pool(name="ps_row_a", bufs=2, space="PSUM") as prow_a,
                tc.tile_pool(name="ps_bc_a", bufs=2, space="PSUM") as pbc_a,
                tc.tile_pool(name="ps_gemm_a", bufs=3, space="PSUM") as pg_a,
                tc.tile_pool(name="st1", bufs=2) as s1pool,
            ):
                x1f8 = [s1pool.tile([128, 2 * TL], F8E4, tag=f"x1f{a}",
                                    name=f"x1f{a}", bufs=1)
                        for a in range(DT // 2)]
                x1T = layernorm_cm(xT_sb, 0, "x1T", prow_a, pbc_a, s1pool,
                                   BF16)
                for j in range(DT):
                    nc.vector.tensor_copy(
                        x1f8[j // 2][:, (j % 2) * TL:(j % 2 + 1) * TL],
                        x1T[j][:])

                def qk_slab(m):
                    # one output slab of the qk GEMM (m<8: q, m>=8: k)
                    slab = wpool.tile([128, DT * 128], F8E4, tag="qkslab",
                                      name="qkslab")
                    nc.sync.dma_start(
                        slab[:], wqk_d[:, m * 1024:(m + 1) * 1024])
                    ps = pg_a.tile([128, TL], F32, tag="gemm", name="qk")
                    for kp in range(DT // 2):
                        _mm8(nc, ps[:],
                             _pair(slab[:, kp * 256:(kp + 1) * 256]),
                             _pair(x1f8[kp][:]), kp == 0, kp == DT // 2 - 1)
                    if m < DT:
                        nc.vector.tensor_scalar_mul(qT_sb[m][:], ps[:],
                                                    1.0 / WS)
                    else:
                        ksb = s1pool.tile([128, TL], F8E4, tag="kevac",
                                          name="kevac", bufs=1)
                        nc.vector.tensor_scalar_mul(ksb[:], ps[:], 1.0 / WS)
                        mk = m - DT
                        nc.gpsimd.dma_start(
                            k_in_ap(mk // 4)[(mk % 4) * 128:
                                             (mk % 4 + 1) * 128, :], ksb[:])

                def v_half(nn):
                    # v GEMM (TM) + pad ones; one half -> bounce buffer
                    wv_slabs = []
                    for kk in range(DT):
                        t = s1pool.tile([128, 512], BF16, tag=f"wv{kk}",
                                        name=f"wv{kk}", bufs=1)
                        nc.sync.dma_start(
                            t[:], wv_d[:, kk * 1024 + nn * 512:
                                       kk * 1024 + (nn + 1) * 512])
                        wv_slabs.append(t)
                    for mt in range(NT):         # 4 token Mtiles
                        ps = pg_a.tile([128, 512], F32, tag="gemm",
                                       name="vps")
                        for kk in range(DT):
                            _mm(nc, ps[:],
                                x1T[kk][:, mt * 128:(mt + 1) * 128],
                                wv_slabs[kk][:], kk == 0, kk == DT - 1)
                        vp = s1pool.tile([128, 520], BF16, tag="vpad",
                                         name="vpad", bufs=2)
                        nc.vector.memset(vp[:], 1.0)
                        dst = vp[:].rearrange("p (h c) -> p h c", c=65)
                        nc.vector.tensor_copy(
                            dst[:, :, 0:64],
                            ps[:].rearrange("p (h c) -> p h c", c=64))
                        nc.gpsimd.dma_start(
                            v_in_ap(nn).rearrange(
                                "p (q c) -> p q c",
                                c=520)[:, :, mt * 130:(mt + 1) * 130],
                            vp[:].rearrange("p (q c) -> p q c", c=130))

                def fire_ag(buf_in, buf_out):
                    nc.gpsimd.collective_compute(
                        "AllGather", ALU.bypass, replica_groups=rg_kv,
                        ins=[buf_in[:].opt()], outs=[buf_out[:].opt()])

                # four small AGs, each fired the moment its producer
                # GEMM finishes — they pipeline on the collective fabric
                for m in range(DT, DT + 4):
                    qk_slab(m)
                fire_ag(k_in[0], k_out[0])
                v_half(0)
                fire_ag(v_in[0], v_out[0])
                for m in range(DT + 4, 2 * DT):
                    qk_slab(m)
                fire_ag(k_in[1], k_out[1])
                v_half(1)
                fire_ag(v_in[1], v_out[1])
                emit_late_consts()
                ones_sb = lateconst["ones"]
                gsel_sb = lateconst["gsel"]
                sel16_sb = lateconst["sel16"]
                eye_sb = lateconst["eye"]
                utri_sb = lateconst["utri"]
                wpfx_sb = lateconst["wpfx"]
                brn_sb = lateconst["brn"]
                be2_sb = lateconst["be2"]
                wrn_sb = lateconst["wrn"]
                bproj_sb = lateconst["bproj"]
                be1_sb = lateconst["be1"]
                bmlp1_sb = lateconst["bmlp1"]
                bmlp2_sb = lateconst["bmlp2"]
                for m in range(DT):
                    qk_slab(m)

            # ---------- attention (2-head interleave, FD-1024 exp) ----------
            aoT = [ppool.tile([128, TL], BF16, tag=f"aoT{p}",
                               name=f"aoT{p}") for p in range(DT)]
            aoRaw = [ppool.tile([128, TL], BF16, tag=f"aoR{p}",
                                name=f"aoR{p}") for p in range(DT)]
            den16 = ppool.tile([2 * DT, TL], F32, tag="den16", name="den16")
            with (
                tc.tile_pool(name="ps_s2", bufs=2, space="PSUM") as ps_s2,
                tc.tile_pool(name="ps_bank", bufs=4, space="PSUM") as ps_bank,
                tc.tile_pool(name="attn", bufs=2) as apool,
                tc.tile_pool(name="vsb", bufs=2) as vpool,
                tc.tile_pool(name="ssb", bufs=3) as spool_s,
            ):
                for p in range(DT):              # head pair
                    hf, pq = p // 4, p % 4       # kv half, pair in half
                    kp = []
                    vt = []
                    for r in range(GRP):
                        kt_ = apool.tile([128, TL], F8E4, tag=f"kp{r}",
                                         name=f"kp{r}")
                        nc.sync.dma_start(kt_[:], k_out_ap(hf, r, pq))
                        kp.append(kt_)
                        vt_ = vpool.tile([128, 520], BF16, tag=f"vt{r}",
                                         name=f"vt{r}")
                        nc.gpsimd.dma_start(vt_[:], v_out_ap(hf, r, pq))
                        vt.append(vt_)
                    ao_ps = [ps_bank.tile([128, TL], F32, tag="bank",
                                          name=f"ao{hh}") for hh in range(2)]
                    steps = [(beat, hh) for beat in range(8)
                             for hh in range(2)]

                    def emit_qk(beat, hh):
                        po = 64 * hh
                        s2 = ps_s2.tile([128, 2 * TL], F32, tag="s2",
                                        name="s2")
                        for u in range(2):
                            kt = 2 * beat + u
                            r, cc = kt // 4, kt % 4
                            _mm(nc, s2[:, u * TL:(u + 1) * TL],
                                kp[r][po:po + 64,
                                      cc * 128:(cc + 1) * 128],
                                qT_sb[p][po:po + 64, :], True, True)
                        return s2

                    s2_next = emit_qk(*steps[0])
                    for idx, (beat, hh) in enumerate(steps):
                        s2 = s2_next
                        if idx + 1 < len(steps):
                            # emit next step's qk BEFORE the exp-dependent
                            # av MMs so the PE queue never stalls on ACT
                            s2_next = emit_qk(*steps[idx + 1])
                        s_sb = spool_s.tile([128, 2 * TL], BF16,
                                            tag="ssb", name="ssb")
                        nc.scalar.activation(s_sb[:], s2[:], AF.Exp,
                                             scale=0.125)
                        for u in range(2):
                            kt = 2 * beat + u
                            r, cc = kt // 4, kt % 4
                            _mm(nc, ao_ps[hh][0:65, :],
                                vt[r][:, cc * 130 + 65 * hh:
                                      cc * 130 + 65 * hh + 65],
                                s_sb[:, u * TL:(u + 1) * TL],
                                kt == 0, kt == 15)
                    for hh in range(2):
                        dtmp = spool.tile([1, TL], F32, tag="lnscr",
                                          name="dtmp", bufs=2)
                        nc.vector.tensor_copy(dtmp[:], ao_ps[hh][64:65, :])
                        nc.scalar.dma_start(
                            den16[2 * p + hh:2 * p + hh + 1, :], dtmp[:])
                        nc.vector.tensor_copy(
                            aoRaw[p][64 * hh:64 * hh + 64, :],
                            ao_ps[hh][0:64, :])

                # batched softmax denominators -> one fast reciprocal
                rec16 = spool.tile([2 * DT, TL], F32, tag="rec16",
                                   name="rec16", bufs=1)
                nc.vector.reciprocal_approx_fast(rec16[:], den16[:])
                rec16b = spool.tile([2 * DT, TL], BF16, tag="rec16b",
                                    name="rec16b", bufs=1)
                nc.vector.tensor_copy(rec16b[:], rec16[:])
                for p in range(DT):
                    bc_ps = ps_bank.tile([128, TL], F32, tag="bank",
                                         name="aobc")
                    _mm(nc, bc_ps[:], sel16_sb[:, p * 128:(p + 1) * 128],
                        rec16b[:], True, True)
                    nc.vector.tensor_tensor(aoT[p][:], aoRaw[p][:],
                                            bc_ps[:], ALU.mult)

            xres = []
            with (
                tc.tile_pool(name="ps_row_c", bufs=2, space="PSUM") as prow_c,
                tc.tile_pool(name="ps_bc_c", bufs=2, space="PSUM") as pbc_c,
                tc.tile_pool(name="ps_gemm_c", bufs=3, space="PSUM") as pg_c,
            ):
                # ---------- proj + residual ----------
                for m in range(DT):
                    slab = wpool.tile([128, DT * 128], BF16, tag="projslab",
                                      name="projslab")
                    nc.sync.dma_start(
                        slab[:], wproj_d[:, m * 1024:(m + 1) * 1024])
                    ps = pg_c.tile([128, TL], F32, tag="gemm", name="proj")
                    for kk in range(DT):
                        _mm(nc, ps[:], slab[:, kk * 128:(kk + 1) * 128],
                            aoT[kk][:], kk == 0, kk == DT - 1)
                    xr = ppool.tile([128, TL], F32, tag=f"xres{m}",
                                    name=f"xres{m}")
                    nc.vector.scalar_tensor_tensor(
                        xr[:], ps[:], bproj_sb[:, m:m + 1], xT_sb[m][:],
                        ALU.add, ALU.add)
                    xres.append(xr)

                # ---------- LN2 ----------
                x2T = layernorm_cm(xres, 1, "x2T", prow_c, pbc_c, ppool)


                # ---------- router (shared route+noise weight tile) -------
                logit_ps = prow_c.tile([E, TL], F32, tag="row",
                                       name="logit")
                for j in range(DT):
                    _mm(nc, logit_ps[:],
                        wrn_sb[:, j * 2 * E:j * 2 * E + E], x2T[j][:],
                        j == 0, j == DT - 1)
                nlin_ps = prow_c.tile([E, TL], F32, tag="row", name="nlin")
                for j in range(DT):
                    _mm(nc, nlin_ps[:],
                        wrn_sb[:, j * 2 * E + E:(j + 1) * 2 * E], x2T[j][:],
                        j == 0, j == DT - 1)
                logits = spool.tile([E, TL], F32, tag="logits", name="logits",
                                    bufs=1)
                nc.vector.tensor_scalar(logits[:], logit_ps[:],
                                        brn_sb[:, 0:1], None, ALU.add)
                spe = spool.tile([E, TL], BF16, tag="softpe", name="softpe",
                                 bufs=1)
                nc.scalar.activation(spe[:], nlin_ps[:], AF.Exp,
                                     bias=brn_sb[:, 1:2])
                spe1 = spool.tile([E, TL], BF16, tag="softpe1",
                                  name="softpe1", bufs=1)
                nc.vector.tensor_scalar_add(spe1[:], spe[:], 1.0)
                sp = spool.tile([E, TL], BF16, tag="softp", name="softp",
                                bufs=1)
                nc.scalar.activation(sp[:], spe1[:], AF.Ln)
                noiseT_sb = spool.tile([E, TL], BF16, tag="noiseTs",
                                       name="noiseTs", bufs=1)
                nc.sync.dma_start(noiseT_sb[:], noiseT_d[:])
                nsp = spool.tile([E, TL], BF16, tag="nsp", name="nsp", bufs=1)
                nc.vector.tensor_tensor(nsp[:], noiseT_sb[:], sp[:],
                                        ALU.mult)
                noisy_cm = spool.tile([E, TL], F32, tag="noisycm",
                                      name="noisycm", bufs=1)
                nc.vector.tensor_tensor(noisy_cm[:], nsp[:], logits[:],
                                        ALU.add)

                # ---------- top-2 gates (TM) ----------
                noisy8 = ppool.tile([128, 8 * NT], F32, tag="noisy8",
                                    name="noisy8")
                nc.vector.memset(noisy8[:], -1e30)
                m8 = ppool.tile([128, 8 * NT], F32, tag="m8", name="m8")
                gate = ppool.tile([128, E * NT], F32, tag="gate", name="gate")
                mask = ppool.tile([128, E * NT], F32, tag="mask", name="mask")
                geT = ppool.tile([E, TL], BF16, tag="geT", name="geT")
                cnt_sb = ppool.tile([1, NT * E], F32, tag="cntsb",
                                    name="cntsb")
                for j in range(NT):
                    tr_ps = pbc_c.tile([128, E], F32, tag="bc", name="ntr")
                    nc.tensor.matmul(tr_ps[:],
                                     noisy_cm[:, j * 128:(j + 1) * 128],
                                     eye_sb[0:E, 0:E], is_transpose=True,
                                     start=True, stop=True)
                    nc.vector.tensor_copy(noisy8[:, 8 * j:8 * j + E],
                                          tr_ps[:])
                # counts-first: fire the capacity AG before the gate math
                diffs = spool.tile([128, NT], F32, tag="diffs",
                                   name="diffs", bufs=1)
                for j in range(NT):
                    nm = noisy8[:, 8 * j:8 * j + E]
                    nc.vector.max(m8[:, 8 * j:8 * j + 8],
                                  noisy8[:, 8 * j:8 * j + 8])
                    v2 = m8[:, 8 * j + 1:8 * j + 2]
                    msk = mask[:, E * j:E * (j + 1)]
                    nc.vector.tensor_scalar(msk, nm, v2, None, ALU.is_ge)
                    nc.vector.tensor_tensor(diffs[:, j:j + 1], v2,
                                            m8[:, 8 * j:8 * j + 1],
                                            ALU.subtract)
                    cps = prow_c.tile([1, E], F32, tag="row", name="cnt")
                    _mm(nc, cps[:], ones_sb[:, 0:1], msk, True, True, F32)
                    nc.vector.tensor_copy(cnt_sb[0:1, E * j:E * (j + 1)],
                                          cps[:])
                p2all = spool.tile([128, NT], F32, tag="p2all",
                                   name="p2all", bufs=1)
                nc.scalar.activation(p2all[:], diffs[:], AF.Exp)

                # total counts -> all-gather
                tot = spool.tile([1, E], F32, tag="cnttot", name="cnttot",
                                 bufs=1)
                nc.vector.tensor_tensor(tot[:], cnt_sb[0:1, 0:E],
                                        cnt_sb[0:1, E:2 * E], ALU.add)
                nc.vector.tensor_tensor(tot[:], tot[:],
                                        cnt_sb[0:1, 2 * E:3 * E], ALU.add)
                nc.vector.tensor_tensor(tot[:], tot[:],
                                        cnt_sb[0:1, 3 * E:4 * E], ALU.add)
                nc.gpsimd.dma_start(cnt_in[:], tot[:])
                nc.gpsimd.collective_compute(
                    "AllGather", ALU.bypass, replica_groups=rg_all,
                    ins=[cnt_in[:].opt()], outs=[cnt_out[:].opt()])

                # gate values (overlap the counts AG)
                for j in range(NT):
                    nm = noisy8[:, 8 * j:8 * j + E]
                    v1 = m8[:, 8 * j:8 * j + 1]
                    msk = mask[:, E * j:E * (j + 1)]
                    oh1 = spool.tile([128, E], F32, tag="oh1", name="oh1")
                    nc.vector.tensor_scalar(oh1[:], nm, v1, None, ALU.is_ge)
                    oh2 = spool.tile([128, E], F32, tag="oh2", name="oh2")
                    nc.vector.tensor_tensor(oh2[:], msk, oh1[:],
                                            ALU.subtract)
                    p2 = p2all[:, j:j + 1]
                    dden = spool.tile([128, 1], F32, tag="dden", name="dden")
                    nc.vector.tensor_scalar_add(dden[:], p2, 1.0)
                    rd = spool.tile([128, 1], F32, tag="rd", name="rd")
                    nc.vector.reciprocal(rd[:], dden[:])
                    gnum = spool.tile([128, E], F32, tag="gnum", name="gnum")
                    nc.vector.tensor_scalar(gnum[:], oh2[:], p2, None,
                                            ALU.mult)
                    gnum2 = spool.tile([128, E], F32, tag="gnum2",
                                       name="gnum2")
                    nc.vector.tensor_tensor(gnum2[:], gnum[:], oh1[:],
                                            ALU.add)
                    nc.vector.tensor_scalar(gate[:, E * j:E * (j + 1)],
                                            gnum2[:], rd[:], None, ALU.mult)

                # ---------- MLP hidden + MoE hidden (overlaps counts AG) ---
                Hm_sb = []
                for m in range(MLPH // 128):
                    slab = wpool.tile([128, DT * 128], BF16, tag="m1slab",
                                      name="m1slab")
                    nc.sync.dma_start(
                        slab[:], wmlp1_d[:, m * 1024:(m + 1) * 1024])
                    ps = pg_c.tile([128, TL], F32, tag="gemm", name="hm")
                    for kk in range(DT):
                        _mm(nc, ps[:], slab[:, kk * 128:(kk + 1) * 128],
                            x2T[kk][:], kk == 0, kk == DT - 1)
                    hm = ppool.tile([128, TL], BF16, tag=f"hm{m}",
                                    name=f"hm{m}")
                    nc.scalar.activation(hm[:], ps[:], AF.Gelu,
                                         bias=bmlp1_sb[:, m:m + 1])
                    Hm_sb.append(hm)
                x2f8 = [ppool.tile([128, 2 * TL], F8E4, tag=f"x2f{a}",
                                   name=f"x2f{a}") for a in range(DT // 2)]
                for j in range(DT):
                    nc.vector.tensor_copy(
                        x2f8[j // 2][:, (j % 2) * TL:(j % 2 + 1) * TL],
                        x2T[j][:])
                Hmoe8 = [ppool.tile([128, 2 * TL], F8E4, tag=f"hmoe8{e}",
                                    name=f"hmoe8{e}") for e in range(E)]
                for e in range(E):
                    for hmi in range(MOEH // 128):
                        me = 2 * e + hmi
                        slab = wpool.tile([128, DT * 128], F8E4, tag="qkslab",
                                          name="e1slab")
                        nc.sync.dma_start(
                            slab[:], we1_d[:, me * 1024:(me + 1) * 1024])
                        ps = pg_c.tile([128, TL], F32, tag="gemm",
                                       name="hmoe")
                        for kp in range(DT // 2):
                            _mm8(nc, ps[:],
                                 _pair(slab[:, kp * 256:(kp + 1) * 256]),
                                 _pair(x2f8[kp][:]), kp == 0,
                                 kp == DT // 2 - 1)
                        nc.scalar.activation(
                            Hmoe8[e][:, hmi * TL:(hmi + 1) * TL],
                            ps[:], AF.Gelu, bias=be1_sb[:, me:me + 1],
                            scale=1.0 / WS)

                # ---------- mlp2 partial of the output GEMM ----------
                # (no dependency on gates/counts AG: fills the AG wait)
                for m in range(DT):
                    slab0 = wpool.tile([128, 20 * 128], BF16, tag="outslab",
                                       name="outslab0")
                    nc.sync.dma_start(
                        slab0[:], wout2_d[:, m * 4096:m * 4096 + 2560])
                    slab1 = wpool.tile([128, 12 * 128], BF16, tag="outslab1",
                                       name="outslab1")
                    nc.sync.dma_start(
                        slab1[:],
                        wout2_d[:, m * 4096 + 2560:(m + 1) * 4096])
                    ps = pg_c.tile([128, TL], F32, tag="gemm", name="om")
                    for kk in range(MLPH // 128):
                        sl = slab0 if kk < 20 else slab1
                        co = kk * 128 if kk < 20 else (kk - 20) * 128
                        _mm(nc, ps[:], sl[:, co:co + 128],
                            Hm_sb[kk][:], kk == 0, kk == MLPH // 128 - 1)
                    # xres += mlp2 partial + bmlp2 (in place, f32)
                    nc.vector.scalar_tensor_tensor(
                        xres[m][:], ps[:], bmlp2_sb[:, m:m + 1], xres[m][:],
                        ALU.add, ALU.add)

                # ---------- ranks / keep / gate_eff ----------
                cntg = spool.tile([NC, E], F32, tag="cntg", name="cntg",
                                  bufs=1)
                nc.gpsimd.dma_start(cntg[:], cnt_out[:])
                off_ps = prow_c.tile([1, E], F32, tag="row", name="off")
                _mm(nc, off_ps[:], wpfx_sb[:], cntg[:], True, True, F32)
                car = spool.tile([1, E * NT], F32, tag="car", name="car",
                                 bufs=1)
                nc.vector.tensor_copy(car[:, 0:E], off_ps[:])
                for j in range(1, NT):
                    nc.vector.tensor_tensor(car[:, E * j:E * (j + 1)],
                                            car[:, E * (j - 1):E * j],
                                            cnt_sb[0:1, E * (j - 1):E * j],
                                            ALU.add)
                ge_tm = ppool.tile([128, E * NT], F32, tag="getm",
                                   name="getm")
                for j in range(NT):
                    rk_ps = pbc_c.tile([128, E], F32, tag="bc", name="rank")
                    _mm(nc, rk_ps[:], utri_sb[:],
                        mask[:, E * j:E * (j + 1)], True, False, F32)
                    _mm(nc, rk_ps[:], ones_sb[0:1, :],
                        car[:, E * j:E * (j + 1)], False, True, F32)
                    keep = spool.tile([128, E], F32, tag="keep", name="keep")
                    nc.vector.tensor_scalar(keep[:], rk_ps[:], float(CAP),
                                            None, ALU.is_lt)
                    nc.vector.tensor_tensor(ge_tm[:, E * j:E * (j + 1)],
                                            gate[:, E * j:E * (j + 1)],
                                            keep[:], ALU.mult)
                for j in range(NT):
                    tr_ps = pbc_c.tile([E, 128], F32, tag="bc", name="getr")
                    nc.tensor.matmul(tr_ps[:], ge_tm[:, E * j:E * (j + 1)],
                                     eye_sb[:, :], is_transpose=True,
                                     start=True, stop=True)
                    nc.vector.tensor_copy(geT[:, j * 128:(j + 1) * 128],
                                          tr_ps[:])

                # gate the MoE hidden
                Hg8 = [ppool.tile([128, 2 * TL], F8E4, tag=f"hg8{e}",
                                  name=f"hg8{e}") for e in range(E)]
                for e in range(E):
                    bc_ps = pbc_c.tile([128, TL], F32, tag="bc", name="gbc")
                    _mm(nc, bc_ps[:], gsel_sb[:, e * 128:(e + 1) * 128],
                        geT[:], True, True)
                    bc_sb = spool.tile([128, TL], BF16, tag="gbcsb",
                                       name="gbcsb", bufs=2)
                    nc.vector.tensor_copy(bc_sb[:], bc_ps[:])
                    for hmi in range(MOEH // 128):
                        nc.vector.tensor_tensor(
                            Hg8[e][:, hmi * TL:(hmi + 1) * TL],
                            Hmoe8[e][:, hmi * TL:(hmi + 1) * TL],
                            bc_sb[:], ALU.mult)

                # ---------- output GEMM: moe + be2 + mlp, fused accum ------
                for m in range(DT):
                    mslab = wpool.tile([128, 8 * 128], F8E4, tag="moeslab",
                                       name="moeslab")
                    nc.sync.dma_start(
                        mslab[:], woutm_d[:, m * 1024:(m + 1) * 1024])
                    ps = pg_c.tile([128, TL], F32, tag="gemm", name="out")
                    for e in range(E):           # we2 pairs per expert
                        _mm8(nc, ps[:],
                             _pair(mslab[:, e * 256:(e + 1) * 256]),
                             _pair(Hg8[e][:]), e == 0, False)
                    _mm(nc, ps[:], be2_sb[:, m * 128:(m + 1) * 128],
                        geT[:], False, True)
                    o = spool.tile([128, TL], F32, tag="outsb", name="outsb",
                                   bufs=2)
                    nc.vector.scalar_tensor_tensor(
                        o[:], ps[:], 1.0 / WS, xres[m][:],
                        ALU.mult, ALU.add)
                    nc.sync.dma_start(out_d[m * 128:(m + 1) * 128, :], o[:])

    nc.compile()
    return nc


def _tile_lhst(w, n_k, n_m):
    # w: [n_k*128, n_m*128] -> [128, n_m, n_k, 128] -> [128, n_m*n_k*128]
    kdim, mdim = w.shape
    return np.ascontiguousarray(
        w.reshape(n_k, 128, n_m, 128).transpose(1, 2, 0, 3)
        .reshape(128, n_m * n_k * 128))


def _prep_inputs(inputs):
    f32 = lambda a: np.ascontiguousarray(np.asarray(a, np.float32))
    bf = lambda a: np.ascontiguousarray(
        np.asarray(a, np.float32).astype(ml_dtypes.bfloat16))
    f8 = lambda a: np.ascontiguousarray(
        np.clip(np.asarray(a, np.float32) * 64.0, -240.0, 240.0)
        .astype(ml_dtypes.float8_e4m3))
    x = f32(inputs["x"]).reshape(T, D)
    noise = f32(inputs["noise"]).reshape(T, E)
    w_qkv = np.asarray(inputs["w_qkv"], np.float32)
    wqkT = w_qkv[:2 * D].T                       # [D, 2048]
    wvT = w_qkv[2 * D:].T                        # [D, D]
    wprojT = np.asarray(inputs["w_proj"], np.float32).T
    we1 = np.asarray(inputs["we1"], np.float32)  # [E, D, MOEH]
    we2 = np.asarray(inputs["we2"], np.float32)  # [E, MOEH, D]
    wmlp1 = np.asarray(inputs["w_mlp1"], np.float32)   # [D, MLPH]
    wmlp2 = np.asarray(inputs["w_mlp2"], np.float32)   # [MLPH, D]

    # we1 slabs: m-index = e*2+hmi over [D, 256] each
    we1_flat = np.concatenate([we1[e] for e in range(E)], 1)  # [D, E*MOEH]
    # wout: per m, 8 we2 tiles (e,hmi) then 32 wmlp2 tiles
    we2_l = we2.reshape(E, 2, 128, DT, 128).transpose(2, 3, 0, 1, 4) \
        .reshape(128, DT, 8, 128)
    wm2_l = wmlp2.reshape(32, 128, DT, 128).transpose(1, 2, 0, 3)

    sel16 = np.zeros((2 * DT, DT * 128), np.float32)
    for p in range(DT):
        sel16[2 * p, p * 128:p * 128 + 64] = 1.0
        sel16[2 * p + 1, p * 128 + 64:(p + 1) * 128] = 1.0

    shared = dict(
        wqk_l=f8(_tile_lhst(wqkT, DT, 16)),
        wv_l=bf(np.ascontiguousarray(
            wvT.reshape(DT, 128, D).transpose(1, 0, 2).reshape(128, DT * D))),
        wproj_l=bf(_tile_lhst(wprojT, DT, DT)),
        wmlp1_l=bf(_tile_lhst(wmlp1, DT, 32)),
        we1_l=f8(_tile_lhst(we1_flat, DT, 8)),
        wout_moe=f8(np.ascontiguousarray(
            we2_l.reshape(128, DT * 8 * 128))),
        wout_mlp2=bf(np.ascontiguousarray(
            wm2_l.reshape(128, DT * 32 * 128))),
        wrn=bf(np.concatenate([inputs["w_route"], inputs["w_noise"]], 1)),
        be2=bf(np.asarray(inputs["be2"], np.float32) * 64.0),
        lng=bf(np.concatenate([np.asarray(inputs["ln1_g"], np.float32),
                               np.asarray(inputs["ln2_g"], np.float32)])
               ).reshape(1, 2 * D),
        lnb=bf(-np.concatenate([np.asarray(inputs["ln1_b"], np.float32),
                                np.asarray(inputs["ln2_b"], np.float32)])
               ).reshape(1, 2 * D),
        bproj=f32(inputs["b_proj"]).reshape(D, 1),
        brn=f32(np.stack([np.asarray(inputs["b_route"], np.float32),
                          np.asarray(inputs["b_noise"], np.float32)], 1)),
        be1=f32(inputs["be1"]).reshape(E * MOEH, 1),
        bmlp1=f32(inputs["b_mlp1"]).reshape(MLPH, 1),
        bmlp2=f32(inputs["b_mlp2"]).reshape(D, 1),
        ones128=np.ones((128, 128), np.float32),
        eye128=np.eye(128, dtype=np.float32),
        utri128=np.triu(np.ones((128, 128), np.float32), 1),
        gsel=np.repeat(np.eye(E, dtype=np.float32),
                       128, 1).astype(ml_dtypes.bfloat16),
        sel16=sel16.astype(ml_dtypes.bfloat16),
        ones128b=np.ones((128, 1), ml_dtypes.bfloat16),
    )
    in_maps = []
    for c in range(NC):
        m = dict(shared)
        m["xT"] = bf(x[c * TL:(c + 1) * TL].T)
        m["noiseT"] = bf(noise[c * TL:(c + 1) * TL].T)
        m["wpfx"] = (np.arange(NC) < c).astype(np.float32).reshape(NC, 1)
        in_maps.append(m)
    return in_maps


def _run(inputs, trace=False):
    if "nc" not in _cache:
        _cache["nc"] = _build()
    nc = _cache["nc"]
    in_maps = _prep_inputs(inputs)
    res = run_bass_kernel_spmd(nc, in_maps, core_ids=list(range(NC)),
                               trace=trace)
    _cache["last_res"] = res
    shards = [res.results[c]["out"] for c in range(NC)]   # each [D, TL]
    out = np.concatenate([np.asarray(s, np.float32).T for s in shards],
                         0).reshape(B, N, D)
    return out.astype(np.float32), res.exec_time_ns


def kernel(**inputs):
    out, _ = _run(inputs, trace=False)
    return out


# revision 29
# speedup vs baseline: 1.0299x; 1.0299x over previous
"""Trainium2 Bass kernel for nn_BlockMoEAdapters (8 NeuronCores, SPMD).

Sharding: tokens (B*N = 4096) split contiguously across 8 cores (512 each).
Cores 0-3 hold batch 0, cores 4-7 batch 1. Attention K/V are all-gathered
(bf16, four quarter-collectives fired as soon as each producer GEMM finishes
so they hide behind the V/Q GEMMs) within each 4-core batch group; MoE
capacity ranks use a tiny 8-core all-gather of per-core expert counts.

Layout: channel-major ([channels, tokens]) on-device for all GEMMs; LayerNorm
stats via ones-matmul partition reductions (f32r for the f32 path); the LN
affine is folded into two rank-2 broadcast matmuls; softmax in [keys, tokens]
orientation with denominators accumulated via a ones-column in V, batched
into one reciprocal_approx_fast at the end of attention; weights host-retiled
into per-output-slab layouts; output shards re-transposed on host.
"""
import sys

for _p in ('/opt/trn_rl_repo',):
    if _p not in sys.path:
        sys.path.append(_p)

import ml_dtypes
import numpy as np

import concourse.bass as bass
import concourse.mybir as mybir
import concourse.tile as tile
from concourse import bacc
from concourse.bass_utils import run_bass_kernel_spmd

F32 = mybir.dt.float32
F32R = mybir.dt.float32r
F8E4 = mybir.dt.float8e4
DR = mybir.MatmulPerfMode.DoubleRow
WS = 64.0   # fp8 weight scale
BF16 = mybir.dt.bfloat16
AF = mybir.ActivationFunctionType
ALU = mybir.AluOpType

B, N, D = 2, 2048, 1024
H, HD = 16, 64
E, TOPK = 4, 2
MOEH, MLPH = 256, 4096
T = B * N
NC = 8
TL = T // NC          # 512 tokens per core
NT = TL // 128        # 4 token tiles
DT = D // 128         # 8 channel tiles
CAP = int(T * TOPK / E * 1.0)   # 2048
GRP = 4               # cores per kv-gather group
EPS = 1e-5

_cache = {}


def _mm(nc, out, lhsT, rhs, start, stop, dt=None):
    if dt is not None:
        lhsT, rhs = lhsT.bitcast(dt), rhs.bitcast(dt)
    nc.tensor.matmul(out, lhsT, rhs, start=start, stop=stop)


def _pair(ap):
    # [128, 2*X] -> [128, 2, X] for DoubleRow operands
    return ap.rearrange("p (two x) -> p two x", two=2)


def _mm8(nc, out, lhsT_pair, rhs_pair, start, stop):
    nc.tensor.matmul(out, lhsT_pair, rhs_pair, start=start, stop=stop,
                     perf_mode=DR)


def _build():
    nc = bacc.Bacc("TRN2", target_bir_lowering=False, debug=False,
                   num_devices=NC)

    def din(name, shape, dt=F32):
        return nc.dram_tensor(name, list(shape), dt, kind="ExternalInput")

    xT_d = din("xT", (D, TL), BF16)
    noiseT_d = din("noiseT", (E, TL), BF16)
    # host-retiled weight slabs (see _prep_inputs for layouts)
    wqk_d = din("wqk_l", (128, 16 * DT * 128), F8E4)
    wv_d = din("wv_l", (128, DT * 1024), BF16)
    wproj_d = din("wproj_l", (128, DT * DT * 128), BF16)
    wmlp1_d = din("wmlp1_l", (128, 32 * DT * 128), BF16)
    we1_d = din("we1_l", (128, 8 * DT * 128), F8E4)
    woutm_d = din("wout_moe", (128, DT * 8 * 128), F8E4)
    wout2_d = din("wout_mlp2", (128, DT * 32 * 128), BF16)
    wrn_d = din("wrn", (D, 2 * E), BF16)         # route cols 0:4, noise 4:8
    be2_d = din("be2", (E, D), BF16)
    lng_d = din("lng", (1, 2 * D), BF16)  # ln1_g ++ ln2_g
    lnb_d = din("lnb", (1, 2 * D), BF16)  # -(ln1_b) ++ -(ln2_b)
    bproj_d = din("bproj", (D, 1))
    brn_d = din("brn", (E, 2))      # col 0 = broute, col 1 = bnoise
    be1_d = din("be1", (E * MOEH, 1))
    bmlp1_d = din("bmlp1", (MLPH, 1))
    bmlp2_d = din("bmlp2", (D, 1))
    ones_d = din("ones128", (128, 128))
    onesb_d = din("ones128b", (128, 1), BF16)
    eye_d = din("eye128", (128, 128))
    utri_d = din("utri128", (128, 128))      # U[s,t] = 1 if s < t
    gsel_d = din("gsel", (E, E * 128), BF16)       # gsel[k, e*128+p] = (k == e)
    sel16_d = din("sel16", (2 * DT, DT * 128), BF16)  # denom selector
    wpfx_d = din("wpfx", (NC, 1))            # per-core: 1 for j < core_id

    out_d = nc.dram_tensor("out", [D, TL], F32, kind="ExternalOutput")

    rg_kv = [[0, 1, 2, 3], [4, 5, 6, 7]]
    rg_all = [list(range(NC))]

    with tile.TileContext(nc) as tc:
        with (
            tc.tile_pool(name="dram", bufs=1, space="DRAM") as dpool,
            tc.tile_pool(name="consts", bufs=1) as cpool,
            tc.tile_pool(name="persist", bufs=1) as ppool,
            tc.tile_pool(name="wslab", bufs=2) as wpool,
            tc.tile_pool(name="scratch", bufs=2) as spool,
        ):
            # ---------- collective bounce buffers (half-split kv) ----------
            k_in = [dpool.tile([D // 2, TL], F8E4, name=f"k_in{h_}")
                    for h_ in range(2)]
            v_in = [dpool.tile([128, 4 * 520], BF16, name=f"v_in{h_}")
                    for h_ in range(2)]
            k_out = [dpool.tile([GRP, D // 2, TL], F8E4, name=f"k_out{h_}")
                     for h_ in range(2)]
            v_out = [dpool.tile([GRP * 128, 4 * 520], BF16,
                                name=f"v_out{h_}") for h_ in range(2)]

            def k_in_ap(h):
                return k_in[h][:]

            def v_in_ap(h):
                return v_in[h][:]

            def k_out_ap(h, r, pq):
                return k_out[h][r, pq * 128:(pq + 1) * 128, :]

            def v_out_ap(h, r, pq):
                return v_out[h][r * 128:(r + 1) * 128,
                                pq * 520:(pq + 1) * 520]
            cnt_in = dpool.tile([1, E], F32, name="cnt_in")
            cnt_out = dpool.tile([NC, E], F32, name="cnt_out")

            # ---------- constants (gpsimd queue) ----------
            def load_const(dram, shape, dt=F32):
                t = cpool.tile(list(shape), dt, tag=dram.name, name=dram.name)
                nc.gpsimd.dma_start(t[:], dram[:])
                return t

            # urgent consts only — the rest load after the AG triggers
            onesb_sb = load_const(onesb_d, (128, 1), BF16)
            lng_sb = load_const(lng_d, (1, 2 * D), BF16)
            lnb_sb = load_const(lnb_d, (1, 2 * D), BF16)
            onesrow = cpool.tile([1, TL], BF16, tag="onesrow",
                                 name="onesrow")
            nc.vector.memset(onesrow[:], 1.0)
            epsc = cpool.tile([1, 1], F32, tag="epsc", name="epsc")
            nc.vector.memset(epsc[:], EPS)

            def load_cols(dram, n, tag):
                # [n*128, 1] dram -> sbuf [128, n] (col j = chunk j)
                t = cpool.tile([128, n], F32, tag=tag, name=tag)
                v = dram[:].rearrange("(a p) b -> a p b", p=128)
                for j in range(n):
                    nc.gpsimd.dma_start(t[:, j:j + 1], v[j])
                return t

            lateconst = {}

            def emit_late_consts():
                lateconst["ones"] = load_const(ones_d, (128, 128))
                lateconst["gsel"] = load_const(gsel_d, (E, E * 128), BF16)
                lateconst["sel16"] = load_const(sel16_d, (2 * DT, DT * 128),
                                                BF16)
                lateconst["eye"] = load_const(eye_d, (128, 128))
                lateconst["utri"] = load_const(utri_d, (128, 128))
                lateconst["wpfx"] = load_const(wpfx_d, (NC, 1))
                lateconst["brn"] = load_const(brn_d, (E, 2))
                lateconst["be2"] = load_const(be2_d, (E, D), BF16)
                wrn_sb = cpool.tile([128, DT * 2 * E], BF16, tag="wrn",
                                    name="wrn")
                for j in range(DT):
                    nc.gpsimd.dma_start(
                        wrn_sb[:, j * 2 * E:(j + 1) * 2 * E],
                        wrn_d[j * 128:(j + 1) * 128, :])
                lateconst["wrn"] = wrn_sb
                lateconst["bproj"] = load_cols(bproj_d, DT, "bproj")
                lateconst["be1"] = load_cols(be1_d, E * MOEH // 128, "be1")
                lateconst["bmlp1"] = load_cols(bmlp1_d, MLPH // 128, "bmlp1")
                lateconst["bmlp2"] = load_cols(bmlp2_d, DT, "bmlp2")

            # ---------- load x (CM, bf16 for GEMM-side, f32 kept in xres) ---
            xT_sb = []
            for j in range(DT):
                t = ppool.tile([128, TL], BF16, tag=f"xT{j}", name=f"xT{j}")
                nc.sync.dma_start(t[:], xT_d[j * 128:(j + 1) * 128, :])
                xT_sb.append(t)

            # ---------- LayerNorm in CM; bf16 output tiles ----------
            # out = (x * (g_r * rsig_t)) - (g_r * mu_t * rsig_t - b_r)
            # via two rank-1/2 broadcast matmuls into PSUM.
            def layernorm_cm(xtiles, lnrow, outtag, prow, pbc, opool,
                             xdt=F32, owrite=None):
                ones_col = onesb_sb[:, 0:1]
                musum = prow.tile([1, TL], F32, tag="row", name="musum")
                sqsum = prow.tile([1, TL], F32, tag="row", name="sqsum")
                for j in range(DT):
                    if xdt == BF16:
                        xb = xtiles[j]
                    else:
                        xb = spool.tile([128, TL], BF16, tag="lnxb",
                                        name="lnxb", bufs=2)
                        nc.vector.tensor_copy(xb[:], xtiles[j][:])
                    _mm(nc, musum[:], ones_col, xb[:], j == 0, j == DT - 1)
                    sq = spool.tile([128, TL], BF16, tag="lnsq", name="lnsq",
                                    bufs=2)
                    nc.vector.tensor_tensor(sq[:], xb[:], xb[:], ALU.mult)
                    _mm(nc, sqsum[:], ones_col, sq[:], j == 0, j == DT - 1)
                mu = spool.tile([1, TL], F32, tag="lnmu", name="lnmu", bufs=1)
                nc.vector.tensor_scalar_mul(mu[:], musum[:], 1.0 / D)
                msq = spool.tile([1, TL], F32, tag="lnscr", name="lnmsq",
                                 bufs=2)
                nc.vector.tensor_tensor(msq[:], mu[:], mu[:], ALU.mult)
                vare = spool.tile([1, TL], F32, tag="lnscr", name="lnvare",
                                  bufs=2)
                nc.vector.scalar_tensor_tensor(vare[:], sqsum[:], 1.0 / D,
                                               msq[:], ALU.mult, ALU.subtract)
                sd = spool.tile([1, TL], F32, tag="lnscr", name="lnsd",
                                bufs=2)
                nc.scalar.activation(sd[:], vare[:], AF.Sqrt, bias=epsc[0:1, 0:1])
                rsig = spool.tile([1, TL], F32, tag="lnrsig", name="lnrsig",
                                  bufs=1)
                nc.vector.reciprocal_approx_fast(rsig[:], sd[:])
                rsigb = spool.tile([1, TL], BF16, tag="lnrsigb",
                                   name="lnrsigb", bufs=1)
                nc.vector.tensor_copy(rsigb[:], rsig[:])
                murs = spool.tile([1, TL], BF16, tag="lnmurs", name="lnmurs",
                                  bufs=1)
                nc.vector.tensor_tensor(murs[:], mu[:], rsig[:], ALU.mult)
                outs = []
                for j in range(DT):
                    o0 = lnrow * D + j * 128
                    gj = lng_sb[0:1, o0:o0 + 128]
                    bj = lnb_sb[0:1, o0:o0 + 128]
                    grs_ps = pbc.tile([128, TL], F32, tag="bc", name="grs")
                    _mm(nc, grs_ps[:], gj, rsigb[:], True, True)
                    gmb_ps = pbc.tile([128, TL], F32, tag="bc", name="gmb")
                    _mm(nc, gmb_ps[:], gj, murs[:], True, False)
                    _mm(nc, gmb_ps[:], bj, onesrow[:], False, True)
                    # gmb = g*mu*rsig - b  (lnb host-negated)
                    t1 = spool.tile([128, TL], F32, tag="lnt1", name="lnt1",
                                    bufs=2)
                    nc.vector.tensor_tensor(t1[:], xtiles[j][:], grs_ps[:],
                                            ALU.mult)
                    if owrite is not None:
                        o = owrite(j)
                    else:
                        ot = opool.tile([128, TL], BF16, tag=f"{outtag}{j}",
                                        name=f"{outtag}{j}", bufs=1)
                        o = ot[:]
                        outs.append(ot)
                    nc.vector.tensor_tensor(o, t1[:], gmb_ps[:],
                                            ALU.subtract)
                return outs

            qT_sb = [ppool.tile([128, TL], F8E4, tag=f"qT{m}",
                                name=f"qT{m}") for m in range(DT)]

            with (
                tc.tile_pool(name="ps_row_a", bufs=2, space="PSUM") as prow_a,
                tc.tile_pool(name="ps_bc_a", bufs=2, space="PSUM") as pbc_a,
                tc.tile_pool(name="ps_gemm_a", bufs=3, space="PSUM") as pg_a,
                tc.tile_pool(name="st1", bufs=2) as s1pool,
            ):
                x1f8 = [s1pool.tile([128, 2 * TL], F8E4, tag=f"x1f{a}",
                                    name=f"x1f{a}", bufs=1)
                        for a in range(DT // 2)]
                x1T = layernorm_cm(xT_sb, 0, "x1T", prow_a, pbc_a, s1pool,
                                   BF16)
                for j in range(DT):
                    nc.vector.tensor_copy(
                        x1f8[j // 2][:, (j % 2) * TL:(j % 2 + 1) * TL],
                        x1T[j][:])

                def qk_slab(m):
                    # one output slab of the qk GEMM (m<8: q, m>=8: k)
                    slab = wpool.tile([128, DT * 128], F8E4, tag="qkslab",
                                      name="qkslab")
                    nc.sync.dma_start(
                        slab[:], wqk_d[:, m * 1024:(m + 1) * 1024])
                    ps = pg_a.tile([128, TL], F32, tag="gemm", name="qk")
                    for kp in range(DT // 2):
                        _mm8(nc, ps[:],
                             _pair(slab[:, kp * 256:(kp + 1) * 256]),
                             _pair(x1f8[kp][:]), kp == 0, kp == DT // 2 - 1)
                    if m < DT:
                        nc.vector.tensor_scalar_mul(qT_sb[m][:], ps[:],
                                                    1.0 / WS)
                    else:
                        ksb = s1pool.tile([128, TL], F8E4, tag="kevac",
                                          name="kevac", bufs=1)
                        nc.vector.tensor_scalar_mul(ksb[:], ps[:], 1.0 / WS)
                        mk = m - DT
                        nc.gpsimd.dma_start(
                            k_in_ap(mk // 4)[(mk % 4) * 128:
                                             (mk % 4 + 1) * 128, :], ksb[:])

                def v_half(nn):
                    # v GEMM (TM) + pad ones; one half -> bounce buffer
                    wv_slabs = []
                    for kk in range(DT):
                        t = s1pool.tile([128, 512], BF16, tag=f"wv{kk}",
                                        name=f"wv{kk}", bufs=1)
                        nc.sync.dma_start(
                            t[:], wv_d[:, kk * 1024 + nn * 512:
                                       kk * 1024 + (nn + 1) * 512])
                        wv_slabs.append(t)
                    for mt in range(NT):         # 4 token Mtiles
                        ps = pg_a.tile([128, 512], F32, tag="gemm",
                                       name="vps")
                        for kk in range(DT):
                            _mm(nc, ps[:],
                                x1T[kk][:, mt * 128:(mt + 1) * 128],
                                wv_slabs[kk][:], kk == 0, kk == DT - 1)
                        vp = s1pool.tile([128, 520], BF16, tag="vpad",
                                         name="vpad", bufs=2)
                        nc.vector.memset(vp[:], 1.0)
                        dst = vp[:].rearrange("p (h c) -> p h c", c=65)
                        nc.vector.tensor_copy(
                            dst[:, :, 0:64],
                            ps[:].rearrange("p (h c) -> p h c", c=64))
                        nc.gpsimd.dma_start(
                            v_in_ap(nn).rearrange(
                                "p (q c) -> p q c",
                                c=520)[:, :, mt * 130:(mt + 1) * 130],
                            vp[:].rearrange("p (q c) -> p q c", c=130))

                def fire_ag(buf_in, buf_out):
                    nc.gpsimd.collective_compute(
                        "AllGather", ALU.bypass, replica_groups=rg_kv,
                        ins=[buf_in[:].opt()], outs=[buf_out[:].opt()])

                # four small AGs, each fired the moment its producer
                # GEMM finishes — they pipeline on the collective fabric
                for m in range(DT, DT + 4):
                    qk_slab(m)
                fire_ag(k_in[0], k_out[0])
                v_half(0)
                fire_ag(v_in[0], v_out[0])
                for m in range(DT + 4, 2 * DT):
                    qk_slab(m)
                fire_ag(k_in[1], k_out[1])
                v_half(1)
                fire_ag(v_in[1], v_out[1])
                emit_late_consts()
                ones_sb = lateconst["ones"]
                gsel_sb = lateconst["gsel"]
                sel16_sb = lateconst["sel16"]
                eye_sb = lateconst["eye"]
                utri_sb = lateconst["utri"]
                wpfx_sb = lateconst["wpfx"]
                brn_sb = lateconst["brn"]
                be2_sb = lateconst["be2"]
                wrn_sb = lateconst["wrn"]
                bproj_sb = lateconst["bproj"]
                be1_sb = lateconst["be1"]
                bmlp1_sb = lateconst["bmlp1"]
                bmlp2_sb = lateconst["bmlp2"]
                for m in range(DT):
                    qk_slab(m)

            # ---------- attention (2-head interleave, FD-1024 exp) ----------
            aoT = [ppool.tile([128, TL], BF16, tag=f"aoT{p}",
                               name=f"aoT{p}") for p in range(DT)]
            aoRaw = [ppool.tile([128, TL], BF16, tag=f"aoR{p}",
                                name=f"aoR{p}") for p in range(DT)]
            den16 = ppool.tile([2 * DT, TL], F32, tag="den16", name="den16")
            with (
                tc.tile_pool(name="ps_s2", bufs=2, space="PSUM") as ps_s2,
                tc.tile_pool(name="ps_bank", bufs=4, space="PSUM") as ps_bank,
                tc.tile_pool(name="attn", bufs=2) as apool,
                tc.tile_pool(name="vsb", bufs=2) as vpool,
                tc.tile_pool(name="ssb", bufs=3) as spool_s,
            ):
                pend = {}

                def flush_den(pp):
                    # den DMA for pair pp, emitted after pair pp+1's
                    # prefetch loads so it never head-of-line blocks them
                    nc.sync.dma_start(den16[2 * pp:2 * pp + 2, :],
                                      pend.pop(pp)[0:1, :])

                for p in range(DT):              # head pair
                    hf, pq = p // 4, p % 4       # kv half, pair in half
                    kp = []
                    vt = []
                    for r in range(GRP):
                        kt_ = apool.tile([128, TL], F8E4, tag=f"kp{r}",
                                         name=f"kp{r}")
                        nc.sync.dma_start(kt_[:], k_out_ap(hf, r, pq))
                        kp.append(kt_)
                        vt_ = vpool.tile([128, 520], BF16, tag=f"vt{r}",
                                         name=f"vt{r}")
                        nc.gpsimd.dma_start(vt_[:], v_out_ap(hf, r, pq))
                        vt.append(vt_)
                    if p - 1 in pend:
                        flush_den(p - 1)
                    ao_ps = [ps_bank.tile([128, TL], F32, tag="bank",
                                          name=f"ao{hh}") for hh in range(2)]
                    steps = [(beat, hh) for beat in range(8)
                             for hh in range(2)]

                    def emit_qk(beat, hh):
                        po = 64 * hh
                        s2 = ps_s2.tile([128, 2 * TL], F32, tag="s2",
                                        name="s2")
                        for u in range(2):
                            kt = 2 * beat + u
                            r, cc = kt // 4, kt % 4
                            _mm(nc, s2[:, u * TL:(u + 1) * TL],
                                kp[r][po:po + 64,
                                      cc * 128:(cc + 1) * 128],
                                qT_sb[p][po:po + 64, :], True, True)
                        return s2

                    s2_next = emit_qk(*steps[0])
                    for idx, (beat, hh) in enumerate(steps):
                        s2 = s2_next
                        if idx + 1 < len(steps):
                            # emit next step's qk BEFORE the exp-dependent
                            # av MMs so the PE queue never stalls on ACT
                            s2_next = emit_qk(*steps[idx + 1])
                        s_sb = spool_s.tile([128, 2 * TL], BF16,
                                            tag="ssb", name="ssb")
                        nc.scalar.activation(s_sb[:], s2[:], AF.Exp,
                                             scale=0.125)
                        for u in range(2):
                            kt = 2 * beat + u
                            r, cc = kt // 4, kt % 4
                            _mm(nc, ao_ps[hh][0:65, :],
                                vt[r][:, cc * 130 + 65 * hh:
                                      cc * 130 + 65 * hh + 65],
                                s_sb[:, u * TL:(u + 1) * TL],
                                kt == 0, kt == 15)
                    dtmp2 = spool.tile([1, 2 * TL], F32, tag="dtmp2",
                                       name="dtmp2", bufs=1)
                    for hh in range(2):
                        nc.vector.tensor_copy(
                            dtmp2[:, hh * TL:(hh + 1) * TL],
                            ao_ps[hh][64:65, :])
                        nc.vector.tensor_copy(
                            aoRaw[p][64 * hh:64 * hh + 64, :],
                            ao_ps[hh][0:64, :])
                    pend[p] = dtmp2

                flush_den(DT - 1)
                # batched softmax denominators -> one fast reciprocal
                rec16 = spool.tile([2 * DT, TL], F32, tag="rec16",
                                   name="rec16", bufs=1)
                nc.vector.reciprocal_approx_fast(rec16[:], den16[:])
                rec16b = spool.tile([2 * DT, TL], BF16, tag="rec16b",
                                    name="rec16b", bufs=1)
                nc.vector.tensor_copy(rec16b[:], rec16[:])
                for p in range(DT):
                    bc_ps = ps_bank.tile([128, TL], F32, tag="bank",
                                         name="aobc")
                    _mm(nc, bc_ps[:], sel16_sb[:, p * 128:(p + 1) * 128],
                        rec16b[:], True, True)
                    nc.vector.tensor_tensor(aoT[p][:], aoRaw[p][:],
                                            bc_ps[:], ALU.mult)

            xres = []
            with (
                tc.tile_pool(name="ps_row_c", bufs=2, space="PSUM") as prow_c,
                tc.tile_pool(name="ps_bc_c", bufs=2, space="PSUM") as pbc_c,
                tc.tile_pool(name="ps_gemm_c", bufs=3, space="PSUM") as pg_c,
            ):
                # ---------- proj + residual ----------
                for m in range(DT):
                    slab = wpool.tile([128, DT * 128], BF16, tag="projslab",
                                      name="projslab")
                    nc.sync.dma_start(
                        slab[:], wproj_d[:, m * 1024:(m + 1) * 1024])
                    ps = pg_c.tile([128, TL], F32, tag="gemm", name="proj")
                    for kk in range(DT):
                        _mm(nc, ps[:], slab[:, kk * 128:(kk + 1) * 128],
                            aoT[kk][:], kk == 0, kk == DT - 1)
                    xr = ppool.tile([128, TL], F32, tag=f"xres{m}",
                                    name=f"xres{m}")
                    nc.vector.scalar_tensor_tensor(
                        xr[:], ps[:], bproj_sb[:, m:m + 1], xT_sb[m][:],
                        ALU.add, ALU.add)
                    xres.append(xr)

                # ---------- LN2 ----------
                x2T = layernorm_cm(xres, 1, "x2T", prow_c, pbc_c, ppool)


                # ---------- router (shared route+noise weight tile) -------
                logit_ps = prow_c.tile([E, TL], F32, tag="row",
                                       name="logit")
                for j in range(DT):
                    _mm(nc, logit_ps[:],
                        wrn_sb[:, j * 2 * E:j * 2 * E + E], x2T[j][:],
                        j == 0, j == DT - 1)
                nlin_ps = prow_c.tile([E, TL], F32, tag="row", name="nlin")
                for j in range(DT):
                    _mm(nc, nlin_ps[:],
                        wrn_sb[:, j * 2 * E + E:(j + 1) * 2 * E], x2T[j][:],
                        j == 0, j == DT - 1)
                logits = spool.tile([E, TL], F32, tag="logits", name="logits",
                                    bufs=1)
                nc.vector.tensor_scalar(logits[:], logit_ps[:],
                                        brn_sb[:, 0:1], None, ALU.add)
                spe = spool.tile([E, TL], BF16, tag="softpe", name="softpe",
                                 bufs=1)
                nc.scalar.activation(spe[:], nlin_ps[:], AF.Exp,
                                     bias=brn_sb[:, 1:2])
                spe1 = spool.tile([E, TL], BF16, tag="softpe1",
                                  name="softpe1", bufs=1)
                nc.vector.tensor_scalar_add(spe1[:], spe[:], 1.0)
                sp = spool.tile([E, TL], BF16, tag="softp", name="softp",
                                bufs=1)
                nc.scalar.activation(sp[:], spe1[:], AF.Ln)
                noiseT_sb = spool.tile([E, TL], BF16, tag="noiseTs",
                                       name="noiseTs", bufs=1)
                nc.sync.dma_start(noiseT_sb[:], noiseT_d[:])
                nsp = spool.tile([E, TL], BF16, tag="nsp", name="nsp", bufs=1)
                nc.vector.tensor_tensor(nsp[:], noiseT_sb[:], sp[:],
                                        ALU.mult)
                noisy_cm = spool.tile([E, TL], F32, tag="noisycm",
                                      name="noisycm", bufs=1)
                nc.vector.tensor_tensor(noisy_cm[:], nsp[:], logits[:],
                                        ALU.add)

                # ---------- top-2 gates (TM) ----------
                noisy8 = ppool.tile([128, 8 * NT], F32, tag="noisy8",
                                    name="noisy8")
                nc.vector.memset(noisy8[:], -1e30)
                m8 = ppool.tile([128, 8 * NT], F32, tag="m8", name="m8")
                gate = ppool.tile([128, E * NT], F32, tag="gate", name="gate")
                mask = ppool.tile([128, E * NT], F32, tag="mask", name="mask")
                geT = ppool.tile([E, TL], BF16, tag="geT", name="geT")
                cnt_sb = ppool.tile([1, NT * E], F32, tag="cntsb",
                                    name="cntsb")
                for j in range(NT):
                    tr_ps = pbc_c.tile([128, E], F32, tag="bc", name="ntr")
                    nc.tensor.matmul(tr_ps[:],
                                     noisy_cm[:, j * 128:(j + 1) * 128],
                                     eye_sb[0:E, 0:E], is_transpose=True,
                                     start=True, stop=True)
                    nc.vector.tensor_copy(noisy8[:, 8 * j:8 * j + E],
                                          tr_ps[:])
                # counts-first: fire the capacity AG before the gate math
                for j in range(NT):
                    nm = noisy8[:, 8 * j:8 * j + E]
                    nc.vector.max(m8[:, 8 * j:8 * j + 8],
                                  noisy8[:, 8 * j:8 * j + 8])
                    v2 = m8[:, 8 * j + 1:8 * j + 2]
                    msk = mask[:, E * j:E * (j + 1)]
                    nc.vector.tensor_scalar(msk, nm, v2, None, ALU.is_ge)
                    cps = prow_c.tile([1, E], F32, tag="row", name="cnt")
                    _mm(nc, cps[:], ones_sb[:, 0:1], msk, True, True, F32)
                    nc.vector.tensor_copy(cnt_sb[0:1, E * j:E * (j + 1)],
                                          cps[:])

                # total counts -> all-gather
                tot = spool.tile([1, E], F32, tag="cnttot", name="cnttot",
                                 bufs=1)
                nc.vector.tensor_tensor(tot[:], cnt_sb[0:1, 0:E],
                                        cnt_sb[0:1, E:2 * E], ALU.add)
                nc.vector.tensor_tensor(tot[:], tot[:],
                                        cnt_sb[0:1, 2 * E:3 * E], ALU.add)
                nc.vector.tensor_tensor(tot[:], tot[:],
                                        cnt_sb[0:1, 3 * E:4 * E], ALU.add)
                nc.gpsimd.dma_start(cnt_in[:], tot[:])
                nc.gpsimd.collective_compute(
                    "AllGather", ALU.bypass, replica_groups=rg_all,
                    ins=[cnt_in[:].opt()], outs=[cnt_out[:].opt()])

                # gate values (overlap the counts AG)
                for j in range(NT):
                    nm = noisy8[:, 8 * j:8 * j + E]
                    v1 = m8[:, 8 * j:8 * j + 1]
                    v2 = m8[:, 8 * j + 1:8 * j + 2]
                    msk = mask[:, E * j:E * (j + 1)]
                    oh1 = spool.tile([128, E], F32, tag="oh1", name="oh1")
                    nc.vector.tensor_scalar(oh1[:], nm, v1, None, ALU.is_ge)
                    oh2 = spool.tile([128, E], F32, tag="oh2", name="oh2")
                    nc.vector.tensor_tensor(oh2[:], msk, oh1[:],
                                            ALU.subtract)
                    negv1 = spool.tile([128, 1], F32, tag="negv1",
                                       name="negv1")
                    nc.vector.tensor_scalar_mul(negv1[:], v1, -1.0)
                    p2 = spool.tile([128, 1], F32, tag="p2", name="p2")
                    nc.scalar.activation(p2[:], v2, AF.Exp, bias=negv1[:])
                    dden = spool.tile([128, 1], F32, tag="dden", name="dden")
                    nc.vector.tensor_scalar_add(dden[:], p2[:], 1.0)
                    rd = spool.tile([128, 1], F32, tag="rd", name="rd")
                    nc.vector.reciprocal(rd[:], dden[:])
                    gnum = spool.tile([128, E], F32, tag="gnum", name="gnum")
                    nc.vector.tensor_scalar(gnum[:], oh2[:], p2[:], None,
                                            ALU.mult)
                    gnum2 = spool.tile([128, E], F32, tag="gnum2",
                                       name="gnum2")
                    nc.vector.tensor_tensor(gnum2[:], gnum[:], oh1[:],
                                            ALU.add)
                    nc.vector.tensor_scalar(gate[:, E * j:E * (j + 1)],
                                            gnum2[:], rd[:], None, ALU.mult)

                # ---------- MLP hidden + MoE hidden (overlaps counts AG) ---
                Hm_sb = []
                for m in range(MLPH // 128):
                    slab = wpool.tile([128, DT * 128], BF16, tag="m1slab",
                                      name="m1slab")
                    nc.sync.dma_start(
                        slab[:], wmlp1_d[:, m * 1024:(m + 1) * 1024])
                    ps = pg_c.tile([128, TL], F32, tag="gemm", name="hm")
                    for kk in range(DT):
                        _mm(nc, ps[:], slab[:, kk * 128:(kk + 1) * 128],
                            x2T[kk][:], kk == 0, kk == DT - 1)
                    hm = ppool.tile([128, TL], BF16, tag=f"hm{m}",
                                    name=f"hm{m}")
                    nc.scalar.activation(hm[:], ps[:], AF.Gelu,
                                         bias=bmlp1_sb[:, m:m + 1])
                    Hm_sb.append(hm)
                x2f8 = [ppool.tile([128, 2 * TL], F8E4, tag=f"x2f{a}",
                                   name=f"x2f{a}") for a in range(DT // 2)]
                for j in range(DT):
                    nc.vector.tensor_copy(
                        x2f8[j // 2][:, (j % 2) * TL:(j % 2 + 1) * TL],
                        x2T[j][:])
                Hmoe8 = [ppool.tile([128, 2 * TL], F8E4, tag=f"hmoe8{e}",
                                    name=f"hmoe8{e}") for e in range(E)]
                for e in range(E):
                    for hmi in range(MOEH // 128):
                        me = 2 * e + hmi
                        slab = wpool.tile([128, DT * 128], F8E4, tag="qkslab",
                                          name="e1slab")
                        nc.sync.dma_start(
                            slab[:], we1_d[:, me * 1024:(me + 1) * 1024])
                        ps = pg_c.tile([128, TL], F32, tag="gemm",
                                       name="hmoe")
                        for kp in range(DT // 2):
                            _mm8(nc, ps[:],
                                 _pair(slab[:, kp * 256:(kp + 1) * 256]),
                                 _pair(x2f8[kp][:]), kp == 0,
                                 kp == DT // 2 - 1)
                        nc.scalar.activation(
                            Hmoe8[e][:, hmi * TL:(hmi + 1) * TL],
                            ps[:], AF.Gelu, bias=be1_sb[:, me:me + 1],
                            scale=1.0 / WS)

                # ---------- mlp2 partial of the output GEMM ----------
                # (no dependency on gates/counts AG: fills the AG wait)
                for m in range(DT):
                    slab0 = wpool.tile([128, 20 * 128], BF16, tag="outslab",
                                       name="outslab0")
                    nc.sync.dma_start(
                        slab0[:], wout2_d[:, m * 4096:m * 4096 + 2560])
                    slab1 = wpool.tile([128, 12 * 128], BF16, tag="outslab1",
                                       name="outslab1")
                    nc.sync.dma_start(
                        slab1[:],
                        wout2_d[:, m * 4096 + 2560:(m + 1) * 4096])
                    ps = pg_c.tile([128, TL], F32, tag="gemm", name="om")
                    for kk in range(MLPH // 128):
                        sl = slab0 if kk < 20 else slab1
                        co = kk * 128 if kk < 20 else (kk - 20) * 128
                        _mm(nc, ps[:], sl[:, co:co + 128],
                            Hm_sb[kk][:], kk == 0, kk == MLPH // 128 - 1)
                    # xres += mlp2 partial + bmlp2 (in place, f32)
                    nc.vector.scalar_tensor_tensor(
                        xres[m][:], ps[:], bmlp2_sb[:, m:m + 1], xres[m][:],
                        ALU.add, ALU.add)

                # ---------- ranks / keep / gate_eff ----------
                cntg = spool.tile([NC, E], F32, tag="cntg", name="cntg",
                                  bufs=1)
                nc.gpsimd.dma_start(cntg[:], cnt_out[:])
                off_ps = prow_c.tile([1, E], F32, tag="row", name="off")
                _mm(nc, off_ps[:], wpfx_sb[:], cntg[:], True, True, F32)
                car = spool.tile([1, E * NT], F32, tag="car", name="car",
                                 bufs=1)
                nc.vector.tensor_copy(car[:, 0:E], off_ps[:])
                for j in range(1, NT):
                    nc.vector.tensor_tensor(car[:, E * j:E * (j + 1)],
                                            car[:, E * (j - 1):E * j],
                                            cnt_sb[0:1, E * (j - 1):E * j],
                                            ALU.add)
                ge_tm = ppool.tile([128, E * NT], F32, tag="getm",
                                   name="getm")
                for j in range(NT):
                    rk_ps = pbc_c.tile([128, E], F32, tag="bc", name="rank")
                    _mm(nc, rk_ps[:], utri_sb[:],
                        mask[:, E * j:E * (j + 1)], True, False, F32)
                    _mm(nc, rk_ps[:], ones_sb[0:1, :],
                        car[:, E * j:E * (j + 1)], False, True, F32)
                    keep = spool.tile([128, E], F32, tag="keep", name="keep")
                    nc.vector.tensor_scalar(keep[:], rk_ps[:], float(CAP),
                                            None, ALU.is_lt)
                    nc.vector.tensor_tensor(ge_tm[:, E * j:E * (j + 1)],
                                            gate[:, E * j:E * (j + 1)],
                                            keep[:], ALU.mult)
                for j in range(NT):
                    tr_ps = pbc_c.tile([E, 128], F32, tag="bc", name="getr")
                    nc.tensor.matmul(tr_ps[:], ge_tm[:, E * j:E * (j + 1)],
                                     eye_sb[:, :], is_transpose=True,
                                     start=True, stop=True)
                    nc.vector.tensor_copy(geT[:, j * 128:(j + 1) * 128],
                                          tr_ps[:])

                # gate the MoE hidden
                Hg8 = [ppool.tile([128, 2 * TL], F8E4, tag=f"hg8{e}",
                                  name=f"hg8{e}") for e in range(E)]
                for e in range(E):
                    bc_ps = pbc_c.tile([128, TL], F32, tag="bc", name="gbc")
                    _mm(nc, bc_ps[:], gsel_sb[:, e * 128:(e + 1) * 128],
                        geT[:], True, True)
                    bc_sb = spool.tile([128, TL], BF16, tag="gbcsb",
                                       name="gbcsb", bufs=2)
                    nc.vector.tensor_copy(bc_sb[:], bc_ps[:])
                    for hmi in range(MOEH // 128):
                        nc.vector.tensor_tensor(
                            Hg8[e][:, hmi * TL:(hmi + 1) * TL],
                            Hmoe8[e][:, hmi * TL:(hmi + 1) * TL],
                            bc_sb[:], ALU.mult)

                # ---------- output GEMM: moe + be2 + mlp, fused accum ------
                for m in range(DT):
                    mslab = wpool.tile([128, 8 * 128], F8E4, tag="moeslab",
                                       name="moeslab")
                    nc.sync.dma_start(
                        mslab[:], woutm_d[:, m * 1024:(m + 1) * 1024])
                    ps = pg_c.tile([128, TL], F32, tag="gemm", name="out")
                    for e in range(E):           # we2 pairs per expert
                        _mm8(nc, ps[:],
                             _pair(mslab[:, e * 256:(e + 1) * 256]),
                             _pair(Hg8[e][:]), e == 0, False)
                    _mm(nc, ps[:], be2_sb[:, m * 128:(m + 1) * 128],
                        geT[:], False, True)
                    o = spool.tile([128, TL], F32, tag="outsb", name="outsb",
                                   bufs=2)
                    nc.vector.scalar_tensor_tensor(
                        o[:], ps[:], 1.0 / WS, xres[m][:],
                        ALU.mult, ALU.add)
                    nc.sync.dma_start(out_d[m * 128:(m + 1) * 128, :], o[:])

    nc.compile()
    return nc


def _tile_lhst(w, n_k, n_m):
    # w: [n_k*128, n_m*128] -> [128, n_m, n_k, 128] -> [128, n_m*n_k*128]
    kdim, mdim = w.shape
    return np.ascontiguousarray(
        w.reshape(n_k, 128, n_m, 128).transpose(1, 2, 0, 3)
        .reshape(128, n_m * n_k * 128))


def _prep_inputs(inputs):
    f32 = lambda a: np.ascontiguousarray(np.asarray(a, np.float32))
    bf = lambda a: np.ascontiguousarray(
        np.asarray(a, np.float32).astype(ml_dtypes.bfloat16))
    f8 = lambda a: np.ascontiguousarray(
        np.clip(np.asarray(a, np.float32) * 64.0, -240.0, 240.0)
        .astype(ml_dtypes.float8_e4m3))
    x = f32(inputs["x"]).reshape(T, D)
    noise = f32(inputs["noise"]).reshape(T, E)
    w_qkv = np.asarray(inputs["w_qkv"], np.float32)
    wqkT = w_qkv[:2 * D].T                       # [D, 2048]
    wvT = w_qkv[2 * D:].T                        # [D, D]
    wprojT = np.asarray(inputs["w_proj"], np.float32).T
    we1 = np.asarray(inputs["we1"], np.float32)  # [E, D, MOEH]
    we2 = np.asarray(inputs["we2"], np.float32)  # [E, MOEH, D]
    wmlp1 = np.asarray(inputs["w_mlp1"], np.float32)   # [D, MLPH]
    wmlp2 = np.asarray(inputs["w_mlp2"], np.float32)   # [MLPH, D]

    # we1 slabs: m-index = e*2+hmi over [D, 256] each
    we1_flat = np.concatenate([we1[e] for e in range(E)], 1)  # [D, E*MOEH]
    # wout: per m, 8 we2 tiles (e,hmi) then 32 wmlp2 tiles
    we2_l = we2.reshape(E, 2, 128, DT, 128).transpose(2, 3, 0, 1, 4) \
        .reshape(128, DT, 8, 128)
    wm2_l = wmlp2.reshape(32, 128, DT, 128).transpose(1, 2, 0, 3)

    sel16 = np.zeros((2 * DT, DT * 128), np.float32)
    for p in range(DT):
        sel16[2 * p, p * 128:p * 128 + 64] = 1.0
        sel16[2 * p + 1, p * 128 + 64:(p + 1) * 128] = 1.0

    shared = dict(
        wqk_l=f8(_tile_lhst(wqkT, DT, 16)),
        wv_l=bf(np.ascontiguousarray(
            wvT.reshape(DT, 128, D).transpose(1, 0, 2).reshape(128, DT * D))),
        wproj_l=bf(_tile_lhst(wprojT, DT, DT)),
        wmlp1_l=bf(_tile_lhst(wmlp1, DT, 32)),
        we1_l=f8(_tile_lhst(we1_flat, DT, 8)),
        wout_moe=f8(np.ascontiguousarray(
            we2_l.reshape(128, DT * 8 * 128))),
        wout_mlp2=bf(np.ascontiguousarray(
            wm2_l.reshape(128, DT * 32 * 128))),
        wrn=bf(np.concatenate([inputs["w_route"], inputs["w_noise"]], 1)),
        be2=bf(np.asarray(inputs["be2"], np.float32) * 64.0),
        lng=bf(np.concatenate([np.asarray(inputs["ln1_g"], np.float32),
                               np.asarray(inputs["ln2_g"], np.float32)])
               ).reshape(1, 2 * D),
        lnb=bf(-np.concatenate([np.asarray(inputs["ln1_b"], np.float32),
                                np.asarray(inputs["ln2_b"], np.float32)])
               ).reshape(1, 2 * D),
        bproj=f32(inputs["b_proj"]).reshape(D, 1),
        brn=f32(np.stack([np.asarray(inputs["b_route"], np.float32),
                          np.asarray(inputs["b_noise"], np.float32)], 1)),
        be1=f32(inputs["be1"]).reshape(E * MOEH, 1),
        bmlp1=f32(inputs["b_mlp1"]).reshape(MLPH, 1),
        bmlp2=f32(inputs["b_mlp2"]).reshape(D, 1),
        ones128=np.ones((128, 128), np.float32),
        eye128=np.eye(128, dtype=np.float32),
        utri128=np.triu(np.ones((128, 128), np.float32), 1),
        gsel=np.repeat(np.eye(E, dtype=np.float32),
                       128, 1).astype(ml_dtypes.bfloat16),
        sel16=sel16.astype(ml_dtypes.bfloat16),
        ones128b=np.ones((128, 1), ml_dtypes.bfloat16),
    )
    in_maps = []
    for c in range(NC):
        m = dict(shared)
        m["xT"] = bf(x[c * TL:(c + 1) * TL].T)
        m["noiseT"] = bf(noise[c * TL:(c + 1) * TL].T)
        m["wpfx"] = (np.arange(NC) < c).astype(np.float32).reshape(NC, 1)
        in_maps.append(m)
    return in_maps


def _run(inputs, trace=False):
    if "nc" not in _cache:
        _cache["nc"] = _build()
    nc = _cache["nc"]
    in_maps = _prep_inputs(inputs)
    res = run_bass_kernel_spmd(nc, in_maps, core_ids=list(range(NC)),
                               trace=trace)
    _cache["last_res"] = res
    shards = [res.results[c]["out"] for c in range(NC)]   # each [D, TL]
    out = np.concatenate([np.asarray(s, np.float32).T for s in shards],
                         0).reshape(B, N, D)
    return out.astype(np.float32), res.exec_time_ns


def kernel(**inputs):
    out, _ = _run(inputs, trace=False)
    return out


# revision 33
# speedup vs baseline: 1.0475x; 1.0171x over previous
"""Trainium2 Bass kernel for nn_BlockMoEAdapters (8 NeuronCores, SPMD).

Sharding: tokens (B*N = 4096) split contiguously across 8 cores (512 each).
Cores 0-3 hold batch 0, cores 4-7 batch 1. Attention K/V are all-gathered
(bf16, four quarter-collectives fired as soon as each producer GEMM finishes
so they hide behind the V/Q GEMMs) within each 4-core batch group; MoE
capacity ranks use a tiny 8-core all-gather of per-core expert counts.

Layout: channel-major ([channels, tokens]) on-device for all GEMMs; LayerNorm
stats via ones-matmul partition reductions (f32r for the f32 path); the LN
affine is folded into two rank-2 broadcast matmuls; softmax in [keys, tokens]
orientation with denominators accumulated via a ones-column in V, batched
into one reciprocal_approx_fast at the end of attention; weights host-retiled
into per-output-slab layouts; output shards re-transposed on host.
"""
import sys

for _p in ('/opt/trn_rl_repo',):
    if _p not in sys.path:
        sys.path.append(_p)

import ml_dtypes
import numpy as np

import concourse.bass as bass
import concourse.mybir as mybir
import concourse.tile as tile
from concourse import bacc
from concourse.bass_utils import run_bass_kernel_spmd

F32 = mybir.dt.float32
F32R = mybir.dt.float32r
F8E4 = mybir.dt.float8e4
DR = mybir.MatmulPerfMode.DoubleRow
WS = 64.0   # fp8 weight scale
BF16 = mybir.dt.bfloat16
AF = mybir.ActivationFunctionType
ALU = mybir.AluOpType

B, N, D = 2, 2048, 1024
H, HD = 16, 64
E, TOPK = 4, 2
MOEH, MLPH = 256, 4096
T = B * N
NC = 8
TL = T // NC          # 512 tokens per core
NT = TL // 128        # 4 token tiles
DT = D // 128         # 8 channel tiles
CAP = int(T * TOPK / E * 1.0)   # 2048
GRP = 4               # cores per kv-gather group
EPS = 1e-5

_cache = {}


def _mm(nc, out, lhsT, rhs, start, stop, dt=None):
    if dt is not None:
        lhsT, rhs = lhsT.bitcast(dt), rhs.bitcast(dt)
    nc.tensor.matmul(out, lhsT, rhs, start=start, stop=stop)


def _pair(ap):
    # [128, 2*X] -> [128, 2, X] for DoubleRow operands
    return ap.rearrange("p (two x) -> p two x", two=2)


def _mm8(nc, out, lhsT_pair, rhs_pair, start, stop):
    nc.tensor.matmul(out, lhsT_pair, rhs_pair, start=start, stop=stop,
                     perf_mode=DR)


def _build():
    nc = bacc.Bacc("TRN2", target_bir_lowering=False, debug=False,
                   num_devices=NC)

    def din(name, shape, dt=F32):
        return nc.dram_tensor(name, list(shape), dt, kind="ExternalInput")

    xT_d = din("xT", (D, TL), BF16)
    noiseT_d = din("noiseT", (E, TL), BF16)
    # host-retiled weight slabs (see _prep_inputs for layouts)
    wqk_d = din("wqk_l", (128, 16 * DT * 128), F8E4)
    wv_d = din("wv_l", (128, DT * 1024), BF16)
    wproj_d = din("wproj_l", (128, DT * DT * 128), BF16)
    wmlp1_d = din("wmlp1_l", (128, 32 * DT * 128), BF16)
    we1_d = din("we1_l", (128, 8 * DT * 128), F8E4)
    woutm_d = din("wout_moe", (128, DT * 8 * 128), F8E4)
    wout2_d = din("wout_mlp2", (128, DT * 32 * 128), BF16)
    wrn_d = din("wrn", (D, 2 * E), BF16)         # route cols 0:4, noise 4:8
    be2_d = din("be2", (E, D), BF16)
    lng_d = din("lng", (1, 2 * D), BF16)  # ln1_g ++ ln2_g
    lnb_d = din("lnb", (1, 2 * D), BF16)  # -(ln1_b) ++ -(ln2_b)
    bproj_d = din("bproj", (D, 1))
    brn_d = din("brn", (E, 2))      # col 0 = broute, col 1 = bnoise
    be1_d = din("be1", (E * MOEH, 1))
    bmlp1_d = din("bmlp1", (MLPH, 1))
    bmlp2_d = din("bmlp2", (D, 1))
    ones_d = din("ones128", (128, 128))
    onesb_d = din("ones128b", (128, 1), BF16)
    eye_d = din("eye128", (128, 128))
    utri_d = din("utri128", (128, 128))      # U[s,t] = 1 if s < t
    gsel_d = din("gsel", (E, E * 128), BF16)       # gsel[k, e*128+p] = (k == e)
    sel16_d = din("sel16", (2 * DT, DT * 128), BF16)  # denom selector
    wpfx_d = din("wpfx", (NC, 1))            # per-core: 1 for j < core_id

    out_d = nc.dram_tensor("out", [D, TL], F32, kind="ExternalOutput")

    rg_kv = [[0, 1, 2, 3], [4, 5, 6, 7]]
    rg_all = [list(range(NC))]

    with tile.TileContext(nc) as tc:
        with (
            tc.tile_pool(name="dram", bufs=1, space="DRAM") as dpool,
            tc.tile_pool(name="consts", bufs=1) as cpool,
            tc.tile_pool(name="persist", bufs=1) as ppool,
            tc.tile_pool(name="wslab", bufs=2) as wpool,
            tc.tile_pool(name="scratch", bufs=2) as spool,
        ):
            # ---------- collective bounce buffers (half-split kv) ----------
            k_in = [dpool.tile([D // 2, TL], F8E4, name=f"k_in{h_}")
                    for h_ in range(2)]
            v_in = [dpool.tile([128, 4 * 520], BF16, name=f"v_in{h_}")
                    for h_ in range(2)]
            k_out = [dpool.tile([GRP, D // 2, TL], F8E4, name=f"k_out{h_}")
                     for h_ in range(2)]
            v_out = [dpool.tile([GRP * 128, 4 * 520], BF16,
                                name=f"v_out{h_}") for h_ in range(2)]

            def k_in_ap(h):
                return k_in[h][:]

            def v_in_ap(h):
                return v_in[h][:]

            def k_out_ap(h, r, pq):
                return k_out[h][r, pq * 128:(pq + 1) * 128, :]

            def v_out_ap(h, r, pq):
                return v_out[h][r * 128:(r + 1) * 128,
                                pq * 520:(pq + 1) * 520]
            cnt_in = dpool.tile([1, E], F32, name="cnt_in")
            cnt_out = dpool.tile([NC, E], F32, name="cnt_out")

            # ---------- constants (gpsimd queue) ----------
            def load_const(dram, shape, dt=F32):
                t = cpool.tile(list(shape), dt, tag=dram.name, name=dram.name)
                nc.gpsimd.dma_start(t[:], dram[:])
                return t

            # urgent consts only — the rest load after the AG triggers
            onesb_sb = load_const(onesb_d, (128, 1), BF16)
            lng_sb = load_const(lng_d, (1, 2 * D), BF16)
            lnb_sb = load_const(lnb_d, (1, 2 * D), BF16)
            onesrow = cpool.tile([1, TL], BF16, tag="onesrow",
                                 name="onesrow")
            nc.vector.memset(onesrow[:], 1.0)
            epsc = cpool.tile([1, 1], F32, tag="epsc", name="epsc")
            nc.vector.memset(epsc[:], EPS)

            def load_cols(dram, n, tag):
                # [n*128, 1] dram -> sbuf [128, n] (col j = chunk j)
                t = cpool.tile([128, n], F32, tag=tag, name=tag)
                v = dram[:].rearrange("(a p) b -> a p b", p=128)
                for j in range(n):
                    nc.gpsimd.dma_start(t[:, j:j + 1], v[j])
                return t

            lateconst = {}

            def emit_late_consts():
                lateconst["ones"] = load_const(ones_d, (128, 128))
                lateconst["gsel"] = load_const(gsel_d, (E, E * 128), BF16)
                lateconst["sel16"] = load_const(sel16_d, (2 * DT, DT * 128),
                                                BF16)
                lateconst["eye"] = load_const(eye_d, (128, 128))
                lateconst["utri"] = load_const(utri_d, (128, 128))
                lateconst["wpfx"] = load_const(wpfx_d, (NC, 1))
                lateconst["brn"] = load_const(brn_d, (E, 2))
                lateconst["be2"] = load_const(be2_d, (E, D), BF16)
                wrn_sb = cpool.tile([128, DT * 2 * E], BF16, tag="wrn",
                                    name="wrn")
                for j in range(DT):
                    nc.gpsimd.dma_start(
                        wrn_sb[:, j * 2 * E:(j + 1) * 2 * E],
                        wrn_d[j * 128:(j + 1) * 128, :])
                lateconst["wrn"] = wrn_sb
                lateconst["bproj"] = load_cols(bproj_d, DT, "bproj")
                lateconst["be1"] = load_cols(be1_d, E * MOEH // 128, "be1")
                lateconst["bmlp1"] = load_cols(bmlp1_d, MLPH // 128, "bmlp1")
                lateconst["bmlp2"] = load_cols(bmlp2_d, DT, "bmlp2")

            # ---------- load x (CM, bf16 for GEMM-side, f32 kept in xres) ---
            xT_sb = []
            for j in range(DT):
                t = ppool.tile([128, TL], BF16, tag=f"xT{j}", name=f"xT{j}")
                nc.sync.dma_start(t[:], xT_d[j * 128:(j + 1) * 128, :])
                xT_sb.append(t)

            # ---------- LayerNorm in CM; bf16 output tiles ----------
            # out = (x * (g_r * rsig_t)) - (g_r * mu_t * rsig_t - b_r)
            # via two rank-1/2 broadcast matmuls into PSUM.
            def layernorm_cm(xtiles, lnrow, outtag, prow, pbc, opool,
                             xdt=F32, owrite=None):
                ones_col = onesb_sb[:, 0:1]
                musum = prow.tile([1, TL], F32, tag="row", name="musum")
                sqsum = prow.tile([1, TL], F32, tag="row", name="sqsum")
                for j in range(DT):
                    if xdt == BF16:
                        xb = xtiles[j]
                    else:
                        xb = spool.tile([128, TL], BF16, tag="lnxb",
                                        name="lnxb", bufs=2)
                        nc.vector.tensor_copy(xb[:], xtiles[j][:])
                    _mm(nc, musum[:], ones_col, xb[:], j == 0, j == DT - 1)
                    sq = spool.tile([128, TL], BF16, tag="lnsq", name="lnsq",
                                    bufs=2)
                    nc.vector.tensor_tensor(sq[:], xb[:], xb[:], ALU.mult)
                    _mm(nc, sqsum[:], ones_col, sq[:], j == 0, j == DT - 1)
                mu = spool.tile([1, TL], F32, tag="lnmu", name="lnmu", bufs=1)
                nc.vector.tensor_scalar_mul(mu[:], musum[:], 1.0 / D)
                msq = spool.tile([1, TL], F32, tag="lnscr", name="lnmsq",
                                 bufs=2)
                nc.vector.tensor_tensor(msq[:], mu[:], mu[:], ALU.mult)
                vare = spool.tile([1, TL], F32, tag="lnscr", name="lnvare",
                                  bufs=2)
                nc.vector.scalar_tensor_tensor(vare[:], sqsum[:], 1.0 / D,
                                               msq[:], ALU.mult, ALU.subtract)
                lnv = spool.tile([1, TL], F32, tag="lnscr", name="lnlnv",
                                 bufs=2)
                nc.scalar.activation(lnv[:], vare[:], AF.Ln,
                                     bias=epsc[0:1, 0:1])
                rsig = spool.tile([1, TL], F32, tag="lnrsig", name="lnrsig",
                                  bufs=1)
                nc.scalar.activation(rsig[:], lnv[:], AF.Exp, scale=-0.5)
                rsigb = spool.tile([1, TL], BF16, tag="lnrsigb",
                                   name="lnrsigb", bufs=1)
                nc.vector.tensor_copy(rsigb[:], rsig[:])
                murs = spool.tile([1, TL], BF16, tag="lnmurs", name="lnmurs",
                                  bufs=1)
                nc.vector.tensor_tensor(murs[:], mu[:], rsig[:], ALU.mult)
                outs = []
                for j in range(DT):
                    o0 = lnrow * D + j * 128
                    gj = lng_sb[0:1, o0:o0 + 128]
                    bj = lnb_sb[0:1, o0:o0 + 128]
                    grs_ps = pbc.tile([128, TL], F32, tag="bc", name="grs")
                    _mm(nc, grs_ps[:], gj, rsigb[:], True, True)
                    gmb_ps = pbc.tile([128, TL], F32, tag="bc", name="gmb")
                    _mm(nc, gmb_ps[:], gj, murs[:], True, False)
                    _mm(nc, gmb_ps[:], bj, onesrow[:], False, True)
                    # gmb = g*mu*rsig - b  (lnb host-negated)
                    t1 = spool.tile([128, TL], F32, tag="lnt1", name="lnt1",
                                    bufs=2)
                    nc.vector.tensor_tensor(t1[:], xtiles[j][:], grs_ps[:],
                                            ALU.mult)
                    if owrite is not None:
                        o = owrite(j)
                    else:
                        ot = opool.tile([128, TL], BF16, tag=f"{outtag}{j}",
                                        name=f"{outtag}{j}", bufs=1)
                        o = ot[:]
                        outs.append(ot)
                    nc.vector.tensor_tensor(o, t1[:], gmb_ps[:],
                                            ALU.subtract)
                return outs

            qT_sb = [ppool.tile([128, TL], F8E4, tag=f"qT{m}",
                                name=f"qT{m}") for m in range(DT)]

            with (
                tc.tile_pool(name="ps_row_a", bufs=2, space="PSUM") as prow_a,
                tc.tile_pool(name="ps_bc_a", bufs=2, space="PSUM") as pbc_a,
                tc.tile_pool(name="ps_gemm_a", bufs=3, space="PSUM") as pg_a,
                tc.tile_pool(name="st1", bufs=2) as s1pool,
            ):
                x1f8 = [s1pool.tile([128, 2 * TL], F8E4, tag=f"x1f{a}",
                                    name=f"x1f{a}", bufs=1)
                        for a in range(DT // 2)]
                x1T = layernorm_cm(xT_sb, 0, "x1T", prow_a, pbc_a, s1pool,
                                   BF16)
                for j in range(DT):
                    nc.vector.tensor_copy(
                        x1f8[j // 2][:, (j % 2) * TL:(j % 2 + 1) * TL],
                        x1T[j][:])

                def qk_slab(m):
                    # one output slab of the qk GEMM (m<8: q, m>=8: k)
                    slab = wpool.tile([128, DT * 128], F8E4, tag="qkslab",
                                      name="qkslab")
                    nc.sync.dma_start(
                        slab[:], wqk_d[:, m * 1024:(m + 1) * 1024])
                    ps = pg_a.tile([128, TL], F32, tag="gemm", name="qk")
                    for kp in range(DT // 2):
                        _mm8(nc, ps[:],
                             _pair(slab[:, kp * 256:(kp + 1) * 256]),
                             _pair(x1f8[kp][:]), kp == 0, kp == DT // 2 - 1)
                    if m < DT:
                        nc.vector.tensor_scalar_mul(qT_sb[m][:], ps[:],
                                                    1.0 / WS)
                    else:
                        ksb = s1pool.tile([128, TL], F8E4, tag="kevac",
                                          name="kevac", bufs=1)
                        nc.vector.tensor_scalar_mul(ksb[:], ps[:], 1.0 / WS)
                        mk = m - DT
                        nc.gpsimd.dma_start(
                            k_in_ap(mk // 4)[(mk % 4) * 128:
                                             (mk % 4 + 1) * 128, :], ksb[:])

                def v_half(nn):
                    # v GEMM (TM) + pad ones; one half -> bounce buffer
                    wv_slabs = []
                    for kk in range(DT):
                        t = s1pool.tile([128, 512], BF16, tag=f"wv{kk}",
                                        name=f"wv{kk}", bufs=1)
                        nc.sync.dma_start(
                            t[:], wv_d[:, kk * 1024 + nn * 512:
                                       kk * 1024 + (nn + 1) * 512])
                        wv_slabs.append(t)
                    for mt in range(NT):         # 4 token Mtiles
                        ps = pg_a.tile([128, 512], F32, tag="gemm",
                                       name="vps")
                        for kk in range(DT):
                            _mm(nc, ps[:],
                                x1T[kk][:, mt * 128:(mt + 1) * 128],
                                wv_slabs[kk][:], kk == 0, kk == DT - 1)
                        vp = s1pool.tile([128, 520], BF16, tag="vpad",
                                         name="vpad", bufs=2)
                        nc.vector.memset(vp[:], 1.0)
                        dst = vp[:].rearrange("p (h c) -> p h c", c=65)
                        nc.vector.tensor_copy(
                            dst[:, :, 0:64],
                            ps[:].rearrange("p (h c) -> p h c", c=64))
                        nc.gpsimd.dma_start(
                            v_in_ap(nn).rearrange(
                                "p (q c) -> p q c",
                                c=520)[:, :, mt * 130:(mt + 1) * 130],
                            vp[:].rearrange("p (q c) -> p q c", c=130))

                def fire_ag(buf_in, buf_out):
                    nc.gpsimd.collective_compute(
                        "AllGather", ALU.bypass, replica_groups=rg_kv,
                        ins=[buf_in[:].opt()], outs=[buf_out[:].opt()])

                # four small AGs, each fired the moment its producer
                # GEMM finishes — they pipeline on the collective fabric
                for m in range(DT, DT + 4):
                    qk_slab(m)
                fire_ag(k_in[0], k_out[0])
                v_half(0)
                fire_ag(v_in[0], v_out[0])
                for m in range(DT + 4, 2 * DT):
                    qk_slab(m)
                fire_ag(k_in[1], k_out[1])
                v_half(1)
                fire_ag(v_in[1], v_out[1])
                emit_late_consts()
                ones_sb = lateconst["ones"]
                gsel_sb = lateconst["gsel"]
                sel16_sb = lateconst["sel16"]
                eye_sb = lateconst["eye"]
                utri_sb = lateconst["utri"]
                wpfx_sb = lateconst["wpfx"]
                brn_sb = lateconst["brn"]
                be2_sb = lateconst["be2"]
                wrn_sb = lateconst["wrn"]
                bproj_sb = lateconst["bproj"]
                be1_sb = lateconst["be1"]
                bmlp1_sb = lateconst["bmlp1"]
                bmlp2_sb = lateconst["bmlp2"]
                for m in range(DT):
                    qk_slab(m)

            # ---------- attention (2-head interleave, FD-1024 exp) ----------
            aoT = [ppool.tile([128, TL], BF16, tag=f"aoT{p}",
                               name=f"aoT{p}") for p in range(DT)]
            aoRaw = [ppool.tile([128, TL], BF16, tag=f"aoR{p}",
                                name=f"aoR{p}") for p in range(DT)]
            den16 = ppool.tile([2 * DT, TL], F32, tag="den16", name="den16")
            with (
                tc.tile_pool(name="ps_s2", bufs=2, space="PSUM") as ps_s2,
                tc.tile_pool(name="ps_bank", bufs=4, space="PSUM") as ps_bank,
                tc.tile_pool(name="attn", bufs=2) as apool,
                tc.tile_pool(name="vsb", bufs=2) as vpool,
                tc.tile_pool(name="ssb", bufs=3) as spool_s,
            ):
                for p in range(DT):              # head pair
                    hf, pq = p // 4, p % 4       # kv half, pair in half
                    kp = []
                    vt = []
                    for r in range(GRP):
                        kt_ = apool.tile([128, TL], F8E4, tag=f"kp{r}",
                                         name=f"kp{r}")
                        nc.sync.dma_start(kt_[:], k_out_ap(hf, r, pq))
                        kp.append(kt_)
                        vt_ = vpool.tile([128, 520], BF16, tag=f"vt{r}",
                                         name=f"vt{r}")
                        nc.gpsimd.dma_start(vt_[:], v_out_ap(hf, r, pq))
                        vt.append(vt_)
                    ao_ps = [ps_bank.tile([128, TL], F32, tag="bank",
                                          name=f"ao{hh}") for hh in range(2)]
                    steps = [(beat, hh) for beat in range(8)
                             for hh in range(2)]

                    def emit_qk(beat, hh):
                        po = 64 * hh
                        s2 = ps_s2.tile([128, 2 * TL], F32, tag="s2",
                                        name="s2")
                        for u in range(2):
                            kt = 2 * beat + u
                            r, cc = kt // 4, kt % 4
                            _mm(nc, s2[:, u * TL:(u + 1) * TL],
                                kp[r][po:po + 64,
                                      cc * 128:(cc + 1) * 128],
                                qT_sb[p][po:po + 64, :], True, True)
                        return s2

                    s2_next = emit_qk(*steps[0])
                    for idx, (beat, hh) in enumerate(steps):
                        s2 = s2_next
                        if idx + 1 < len(steps):
                            # emit next step's qk BEFORE the exp-dependent
                            # av MMs so the PE queue never stalls on ACT
                            s2_next = emit_qk(*steps[idx + 1])
                        s_sb = spool_s.tile([128, 2 * TL], BF16,
                                            tag="ssb", name="ssb")
                        nc.scalar.activation(s_sb[:], s2[:], AF.Exp,
                                             scale=0.125)
                        for u in range(2):
                            kt = 2 * beat + u
                            r, cc = kt // 4, kt % 4
                            _mm(nc, ao_ps[hh][0:65, :],
                                vt[r][:, cc * 130 + 65 * hh:
                                      cc * 130 + 65 * hh + 65],
                                s_sb[:, u * TL:(u + 1) * TL],
                                kt == 0, kt == 15)
                    for hh in range(2):
                        dtmp = spool.tile([1, TL], F32, tag="lnscr",
                                          name="dtmp", bufs=2)
                        nc.vector.tensor_copy(dtmp[:], ao_ps[hh][64:65, :])
                        nc.scalar.dma_start(
                            den16[2 * p + hh:2 * p + hh + 1, :], dtmp[:])
                        nc.vector.tensor_copy(
                            aoRaw[p][64 * hh:64 * hh + 64, :],
                            ao_ps[hh][0:64, :])

                # batched softmax denominators -> one fast reciprocal
                rec16 = spool.tile([2 * DT, TL], F32, tag="rec16",
                                   name="rec16", bufs=1)
                nc.vector.reciprocal_approx_fast(rec16[:], den16[:])
                rec16b = spool.tile([2 * DT, TL], BF16, tag="rec16b",
                                    name="rec16b", bufs=1)
                nc.vector.tensor_copy(rec16b[:], rec16[:])
                for p in range(DT):
                    bc_ps = ps_bank.tile([128, TL], F32, tag="bank",
                                         name="aobc")
                    _mm(nc, bc_ps[:], sel16_sb[:, p * 128:(p + 1) * 128],
                        rec16b[:], True, True)
                    nc.vector.tensor_tensor(aoT[p][:], aoRaw[p][:],
                                            bc_ps[:], ALU.mult)

            xres = []
            with (
                tc.tile_pool(name="ps_row_c", bufs=2, space="PSUM") as prow_c,
                tc.tile_pool(name="ps_bc_c", bufs=2, space="PSUM") as pbc_c,
                tc.tile_pool(name="ps_gemm_c", bufs=3, space="PSUM") as pg_c,
            ):
                # ---------- proj + residual ----------
                for m in range(DT):
                    slab = wpool.tile([128, DT * 128], BF16, tag="projslab",
                                      name="projslab")
                    nc.sync.dma_start(
                        slab[:], wproj_d[:, m * 1024:(m + 1) * 1024])
                    ps = pg_c.tile([128, TL], F32, tag="gemm", name="proj")
                    for kk in range(DT):
                        _mm(nc, ps[:], slab[:, kk * 128:(kk + 1) * 128],
                            aoT[kk][:], kk == 0, kk == DT - 1)
                    xr = ppool.tile([128, TL], F32, tag=f"xres{m}",
                                    name=f"xres{m}")
                    nc.vector.scalar_tensor_tensor(
                        xr[:], ps[:], bproj_sb[:, m:m + 1], xT_sb[m][:],
                        ALU.add, ALU.add)
                    xres.append(xr)

                # ---------- LN2 ----------
                x2T = layernorm_cm(xres, 1, "x2T", prow_c, pbc_c, ppool)


                # ---------- router (shared route+noise weight tile) -------
                logit_ps = prow_c.tile([E, TL], F32, tag="row",
                                       name="logit")
                for j in range(DT):
                    _mm(nc, logit_ps[:],
                        wrn_sb[:, j * 2 * E:j * 2 * E + E], x2T[j][:],
                        j == 0, j == DT - 1)
                nlin_ps = prow_c.tile([E, TL], F32, tag="row", name="nlin")
                for j in range(DT):
                    _mm(nc, nlin_ps[:],
                        wrn_sb[:, j * 2 * E + E:(j + 1) * 2 * E], x2T[j][:],
                        j == 0, j == DT - 1)
                logits = spool.tile([E, TL], F32, tag="logits", name="logits",
                                    bufs=1)
                nc.vector.tensor_scalar(logits[:], logit_ps[:],
                                        brn_sb[:, 0:1], None, ALU.add)
                spe = spool.tile([E, TL], BF16, tag="softpe", name="softpe",
                                 bufs=1)
                nc.scalar.activation(spe[:], nlin_ps[:], AF.Exp,
                                     bias=brn_sb[:, 1:2])
                spe1 = spool.tile([E, TL], BF16, tag="softpe1",
                                  name="softpe1", bufs=1)
                nc.vector.tensor_scalar_add(spe1[:], spe[:], 1.0)
                sp = spool.tile([E, TL], BF16, tag="softp", name="softp",
                                bufs=1)
                nc.scalar.activation(sp[:], spe1[:], AF.Ln)
                noiseT_sb = spool.tile([E, TL], BF16, tag="noiseTs",
                                       name="noiseTs", bufs=1)
                nc.sync.dma_start(noiseT_sb[:], noiseT_d[:])
                nsp = spool.tile([E, TL], BF16, tag="nsp", name="nsp", bufs=1)
                nc.vector.tensor_tensor(nsp[:], noiseT_sb[:], sp[:],
                                        ALU.mult)
                noisy_cm = spool.tile([E, TL], F32, tag="noisycm",
                                      name="noisycm", bufs=1)
                nc.vector.tensor_tensor(noisy_cm[:], nsp[:], logits[:],
                                        ALU.add)

                # ---------- top-2 gates (TM) ----------
                noisy8 = ppool.tile([128, 8 * NT], F32, tag="noisy8",
                                    name="noisy8")
                nc.vector.memset(noisy8[:], -1e30)
                m8 = ppool.tile([128, 8 * NT], F32, tag="m8", name="m8")
                gate = ppool.tile([128, E * NT], F32, tag="gate", name="gate")
                mask = ppool.tile([128, E * NT], F32, tag="mask", name="mask")
                geT = ppool.tile([E, TL], BF16, tag="geT", name="geT")
                cnt_sb = ppool.tile([1, NT * E], F32, tag="cntsb",
                                    name="cntsb")
                for j in range(NT):
                    tr_ps = pbc_c.tile([128, E], F32, tag="bc", name="ntr")
                    nc.tensor.matmul(tr_ps[:],
                                     noisy_cm[:, j * 128:(j + 1) * 128],
                                     eye_sb[0:E, 0:E], is_transpose=True,
                                     start=True, stop=True)
                    nc.vector.tensor_copy(noisy8[:, 8 * j:8 * j + E],
                                          tr_ps[:])
                # counts-first: fire the capacity AG before the gate math
                diffs = spool.tile([128, NT], F32, tag="diffs",
                                   name="diffs", bufs=1)
                for j in range(NT):
                    nm = noisy8[:, 8 * j:8 * j + E]
                    nc.vector.max(m8[:, 8 * j:8 * j + 8],
                                  noisy8[:, 8 * j:8 * j + 8])
                    v2 = m8[:, 8 * j + 1:8 * j + 2]
                    msk = mask[:, E * j:E * (j + 1)]
                    nc.vector.tensor_scalar(msk, nm, v2, None, ALU.is_ge)
                    nc.vector.tensor_tensor(diffs[:, j:j + 1], v2,
                                            m8[:, 8 * j:8 * j + 1],
                                            ALU.subtract)
                    cps = prow_c.tile([1, E], F32, tag="row", name="cnt")
                    _mm(nc, cps[:], ones_sb[:, 0:1], msk, True, True, F32)
                    nc.vector.tensor_copy(cnt_sb[0:1, E * j:E * (j + 1)],
                                          cps[:])
                p2all = spool.tile([128, NT], F32, tag="p2all",
                                   name="p2all", bufs=1)
                nc.scalar.activation(p2all[:], diffs[:], AF.Exp)

                # total counts -> all-gather
                tot = spool.tile([1, E], F32, tag="cnttot", name="cnttot",
                                 bufs=1)
                nc.vector.tensor_tensor(tot[:], cnt_sb[0:1, 0:E],
                                        cnt_sb[0:1, E:2 * E], ALU.add)
                nc.vector.tensor_tensor(tot[:], tot[:],
                                        cnt_sb[0:1, 2 * E:3 * E], ALU.add)
                nc.vector.tensor_tensor(tot[:], tot[:],
                                        cnt_sb[0:1, 3 * E:4 * E], ALU.add)
                nc.gpsimd.dma_start(cnt_in[:], tot[:])
                nc.gpsimd.collective_compute(
                    "AllGather", ALU.bypass, replica_groups=rg_all,
                    ins=[cnt_in[:].opt()], outs=[cnt_out[:].opt()])

                # gate values (overlap the counts AG)
                for j in range(NT):
                    nm = noisy8[:, 8 * j:8 * j + E]
                    v1 = m8[:, 8 * j:8 * j + 1]
                    msk = mask[:, E * j:E * (j + 1)]
                    oh1 = spool.tile([128, E], F32, tag="oh1", name="oh1")
                    nc.vector.tensor_scalar(oh1[:], nm, v1, None, ALU.is_ge)
                    oh2 = spool.tile([128, E], F32, tag="oh2", name="oh2")
                    nc.vector.tensor_tensor(oh2[:], msk, oh1[:],
                                            ALU.subtract)
                    p2 = p2all[:, j:j + 1]
                    dden = spool.tile([128, 1], F32, tag="dden", name="dden")
                    nc.vector.tensor_scalar_add(dden[:], p2, 1.0)
                    rd = spool.tile([128, 1], F32, tag="rd", name="rd")
                    nc.vector.reciprocal(rd[:], dden[:])
                    gnum = spool.tile([128, E], F32, tag="gnum", name="gnum")
                    nc.vector.tensor_scalar(gnum[:], oh2[:], p2, None,
                                            ALU.mult)
                    gnum2 = spool.tile([128, E], F32, tag="gnum2",
                                       name="gnum2")
                    nc.vector.tensor_tensor(gnum2[:], gnum[:], oh1[:],
                                            ALU.add)
                    nc.vector.tensor_scalar(gate[:, E * j:E * (j + 1)],
                                            gnum2[:], rd[:], None, ALU.mult)

                # ---------- MLP hidden + MoE hidden (overlaps counts AG) ---
                Hm_sb = []
                for m in range(MLPH // 128):
                    slab = wpool.tile([128, DT * 128], BF16, tag="m1slab",
                                      name="m1slab")
                    nc.sync.dma_start(
                        slab[:], wmlp1_d[:, m * 1024:(m + 1) * 1024])
                    ps = pg_c.tile([128, TL], F32, tag="gemm", name="hm")
                    for kk in range(DT):
                        _mm(nc, ps[:], slab[:, kk * 128:(kk + 1) * 128],
                            x2T[kk][:], kk == 0, kk == DT - 1)
                    hm = ppool.tile([128, TL], BF16, tag=f"hm{m}",
                                    name=f"hm{m}")
                    nc.scalar.activation(hm[:], ps[:], AF.Gelu,
                                         bias=bmlp1_sb[:, m:m + 1])
                    Hm_sb.append(hm)
                x2f8 = [ppool.tile([128, 2 * TL], F8E4, tag=f"x2f{a}",
                                   name=f"x2f{a}") for a in range(DT // 2)]
                for j in range(DT):
                    nc.vector.tensor_copy(
                        x2f8[j // 2][:, (j % 2) * TL:(j % 2 + 1) * TL],
                        x2T[j][:])
                Hmoe8 = [ppool.tile([128, 2 * TL], F8E4, tag=f"hmoe8{e}",
                                    name=f"hmoe8{e}") for e in range(E)]
                for e in range(E):
                    for hmi in range(MOEH // 128):
                        me = 2 * e + hmi
                        slab = wpool.tile([128, DT * 128], F8E4, tag="qkslab",
                                          name="e1slab")
                        nc.sync.dma_start(
                            slab[:], we1_d[:, me * 1024:(me + 1) * 1024])
                        ps = pg_c.tile([128, TL], F32, tag="gemm",
                                       name="hmoe")
                        for kp in range(DT // 2):
                            _mm8(nc, ps[:],
                                 _pair(slab[:, kp * 256:(kp + 1) * 256]),
                                 _pair(x2f8[kp][:]), kp == 0,
                                 kp == DT // 2 - 1)
                        nc.scalar.activation(
                            Hmoe8[e][:, hmi * TL:(hmi + 1) * TL],
                            ps[:], AF.Gelu, bias=be1_sb[:, me:me + 1],
                            scale=1.0 / WS)

                # ---------- mlp2 partial of the output GEMM ----------
                # (no dependency on gates/counts AG: fills the AG wait)
                for m in range(DT):
                    slab0 = wpool.tile([128, 20 * 128], BF16, tag="outslab",
                                       name="outslab0")
                    nc.sync.dma_start(
                        slab0[:], wout2_d[:, m * 4096:m * 4096 + 2560])
                    slab1 = wpool.tile([128, 12 * 128], BF16, tag="outslab1",
                                       name="outslab1")
                    nc.sync.dma_start(
                        slab1[:],
                        wout2_d[:, m * 4096 + 2560:(m + 1) * 4096])
                    ps = pg_c.tile([128, TL], F32, tag="gemm", name="om")
                    for kk in range(MLPH // 128):
                        sl = slab0 if kk < 20 else slab1
                        co = kk * 128 if kk < 20 else (kk - 20) * 128
                        _mm(nc, ps[:], sl[:, co:co + 128],
                            Hm_sb[kk][:], kk == 0, kk == MLPH // 128 - 1)
                    # xres += mlp2 partial + bmlp2 (in place, f32)
                    nc.vector.scalar_tensor_tensor(
                        xres[m][:], ps[:], bmlp2_sb[:, m:m + 1], xres[m][:],
                        ALU.add, ALU.add)

                # ---------- ranks / keep / gate_eff ----------
                cntg = spool.tile([NC, E], F32, tag="cntg", name="cntg",
                                  bufs=1)
                nc.gpsimd.dma_start(cntg[:], cnt_out[:])
                off_ps = prow_c.tile([1, E], F32, tag="row", name="off")
                _mm(nc, off_ps[:], wpfx_sb[:], cntg[:], True, True, F32)
                car = spool.tile([1, E * NT], F32, tag="car", name="car",
                                 bufs=1)
                nc.vector.tensor_copy(car[:, 0:E], off_ps[:])
                for j in range(1, NT):
                    nc.vector.tensor_tensor(car[:, E * j:E * (j + 1)],
                                            car[:, E * (j - 1):E * j],
                                            cnt_sb[0:1, E * (j - 1):E * j],
                                            ALU.add)
                ge_tm = ppool.tile([128, E * NT], F32, tag="getm",
                                   name="getm")
                for j in range(NT):
                    rk_ps = pbc_c.tile([128, E], F32, tag="bc", name="rank")
                    _mm(nc, rk_ps[:], utri_sb[:],
                        mask[:, E * j:E * (j + 1)], True, False, F32)
                    _mm(nc, rk_ps[:], ones_sb[0:1, :],
                        car[:, E * j:E * (j + 1)], False, True, F32)
                    keep = spool.tile([128, E], F32, tag="keep", name="keep")
                    nc.vector.tensor_scalar(keep[:], rk_ps[:], float(CAP),
                                            None, ALU.is_lt)
                    nc.vector.tensor_tensor(ge_tm[:, E * j:E * (j + 1)],
                                            gate[:, E * j:E * (j + 1)],
                                            keep[:], ALU.mult)
                for j in range(NT):
                    tr_ps = pbc_c.tile([E, 128], F32, tag="bc", name="getr")
                    nc.tensor.matmul(tr_ps[:], ge_tm[:, E * j:E * (j + 1)],
                                     eye_sb[:, :], is_transpose=True,
                                     start=True, stop=True)
                    nc.vector.tensor_copy(geT[:, j * 128:(j + 1) * 128],
                                          tr_ps[:])

                # gate the MoE hidden
                Hg8 = [ppool.tile([128, 2 * TL], F8E4, tag=f"hg8{e}",
                                  name=f"hg8{e}") for e in range(E)]
                for e in range(E):
                    bc_ps = pbc_c.tile([128, TL], F32, tag="bc", name="gbc")
                    _mm(nc, bc_ps[:], gsel_sb[:, e * 128:(e + 1) * 128],
                        geT[:], True, True)
                    bc_sb = spool.tile([128, TL], BF16, tag="gbcsb",
                                       name="gbcsb", bufs=2)
                    nc.vector.tensor_copy(bc_sb[:], bc_ps[:])
                    for hmi in range(MOEH // 128):
                        nc.vector.tensor_tensor(
                            Hg8[e][:, hmi * TL:(hmi + 1) * TL],
                            Hmoe8[e][:, hmi * TL:(hmi + 1) * TL],
                            bc_sb[:], ALU.mult)

                # ---------- output GEMM: moe + be2 + mlp, fused accum ------
                for m in range(DT):
                    mslab = wpool.tile([128, 8 * 128], F8E4, tag="moeslab",
                                       name="moeslab")
                    nc.sync.dma_start(
                        mslab[:], woutm_d[:, m * 1024:(m + 1) * 1024])
                    ps = pg_c.tile([128, TL], F32, tag="gemm", name="out")
                    for e in range(E):           # we2 pairs per expert
                        _mm8(nc, ps[:],
                             _pair(mslab[:, e * 256:(e + 1) * 256]),
                             _pair(Hg8[e][:]), e == 0, False)
                    _mm(nc, ps[:], be2_sb[:, m * 128:(m + 1) * 128],
                        geT[:], False, True)
                    o = spool.tile([128, TL], F32, tag="outsb", name="outsb",
                                   bufs=2)
                    nc.vector.scalar_tensor_tensor(
                        o[:], ps[:], 1.0 / WS, xres[m][:],
                        ALU.mult, ALU.add)
                    nc.sync.dma_start(out_d[m * 128:(m + 1) * 128, :], o[:])

    nc.compile()
    return nc


def _tile_lhst(w, n_k, n_m):
    # w: [n_k*128, n_m*128] -> [128, n_m, n_k, 128] -> [128, n_m*n_k*128]
    kdim, mdim = w.shape
    return np.ascontiguousarray(
        w.reshape(n_k, 128, n_m, 128).transpose(1, 2, 0, 3)
        .reshape(128, n_m * n_k * 128))


def _prep_inputs(inputs):
    f32 = lambda a: np.ascontiguousarray(np.asarray(a, np.float32))
    bf = lambda a: np.ascontiguousarray(
        np.asarray(a, np.float32).astype(ml_dtypes.bfloat16))
    f8 = lambda a: np.ascontiguousarray(
        np.clip(np.asarray(a, np.float32) * 64.0, -240.0, 240.0)
        .astype(ml_dtypes.float8_e4m3))
    x = f32(inputs["x"]).reshape(T, D)
    noise = f32(inputs["noise"]).reshape(T, E)
    w_qkv = np.asarray(inputs["w_qkv"], np.float32)
    wqkT = w_qkv[:2 * D].T                       # [D, 2048]
    wvT = w_qkv[2 * D:].T                        # [D, D]
    wprojT = np.asarray(inputs["w_proj"], np.float32).T
    we1 = np.asarray(inputs["we1"], np.float32)  # [E, D, MOEH]
    we2 = np.asarray(inputs["we2"], np.float32)  # [E, MOEH, D]
    wmlp1 = np.asarray(inputs["w_mlp1"], np.float32)   # [D, MLPH]
    wmlp2 = np.asarray(inputs["w_mlp2"], np.float32)   # [MLPH, D]

    # we1 slabs: m-index = e*2+hmi over [D, 256] each
    we1_flat = np.concatenate([we1[e] for e in range(E)], 1)  # [D, E*MOEH]
    # wout: per m, 8 we2 tiles (e,hmi) then 32 wmlp2 tiles
    we2_l = we2.reshape(E, 2, 128, DT, 128).transpose(2, 3, 0, 1, 4) \
        .reshape(128, DT, 8, 128)
    wm2_l = wmlp2.reshape(32, 128, DT, 128).transpose(1, 2, 0, 3)

    sel16 = np.zeros((2 * DT, DT * 128), np.float32)
    for p in range(DT):
        sel16[2 * p, p * 128:p * 128 + 64] = 1.0
        sel16[2 * p + 1, p * 128 + 64:(p + 1) * 128] = 1.0

    shared = dict(
        wqk_l=f8(_tile_lhst(wqkT, DT, 16)),
        wv_l=bf(np.ascontiguousarray(
            wvT.reshape(DT, 128, D).transpose(1, 0, 2).reshape(128, DT * D))),
        wproj_l=bf(_tile_lhst(wprojT, DT, DT)),
        wmlp1_l=bf(_tile_lhst(wmlp1, DT, 32)),
        we1_l=f8(_tile_lhst(we1_flat, DT, 8)),
        wout_moe=f8(np.ascontiguousarray(
            we2_l.reshape(128, DT * 8 * 128))),
        wout_mlp2=bf(np.ascontiguousarray(
            wm2_l.reshape(128, DT * 32 * 128))),
        wrn=bf(np.concatenate([inputs["w_route"], inputs["w_noise"]], 1)),
        be2=bf(np.asarray(inputs["be2"], np.float32) * 64.0),
        lng=bf(np.concatenate([np.asarray(inputs["ln1_g"], np.float32),
                               np.asarray(inputs["ln2_g"], np.float32)])
               ).reshape(1, 2 * D),
        lnb=bf(-np.concatenate([np.asarray(inputs["ln1_b"], np.float32),
                                np.asarray(inputs["ln2_b"], np.float32)])
               ).reshape(1, 2 * D),
        bproj=f32(inputs["b_proj"]).reshape(D, 1),
        brn=f32(np.stack([np.asarray(inputs["b_route"], np.float32),
                          np.asarray(inputs["b_noise"], np.float32)], 1)),
        be1=f32(inputs["be1"]).reshape(E * MOEH, 1),
        bmlp1=f32(inputs["b_mlp1"]).reshape(MLPH, 1),
        bmlp2=f32(inputs["b_mlp2"]).reshape(D, 1),
        ones128=np.ones((128, 128), np.float32),
        eye128=np.eye(128, dtype=np.float32),
        utri128=np.triu(np.ones((128, 128), np.float32), 1),
        gsel=np.repeat(np.eye(E, dtype=np.float32),
                       128, 1).astype(ml_dtypes.bfloat16),
        sel16=sel16.astype(ml_dtypes.bfloat16),
        ones128b=np.ones((128, 1), ml_dtypes.bfloat16),
    )
    in_maps = []
    for c in range(NC):
        m = dict(shared)
        m["xT"] = bf(x[c * TL:(c + 1) * TL].T)
        m["noiseT"] = bf(noise[c * TL:(c + 1) * TL].T)
        m["wpfx"] = (np.arange(NC) < c).astype(np.float32).reshape(NC, 1)
        in_maps.append(m)
    return in_maps


def _run(inputs, trace=False):
    if "nc" not in _cache:
        _cache["nc"] = _build()
    nc = _cache["nc"]
    in_maps = _prep_inputs(inputs)
    res = run_bass_kernel_spmd(nc, in_maps, core_ids=list(range(NC)),
                               trace=trace)
    _cache["last_res"] = res
    shards = [res.results[c]["out"] for c in range(NC)]   # each [D, TL]
    out = np.concatenate([np.asarray(s, np.float32).T for s in shards],
                         0).reshape(B, N, D)
    return out.astype(np.float32), res.exec_time_ns


def kernel(**inputs):
    out, _ = _run(inputs, trace=False)
    return out
